# revision 16
# baseline (speedup 1.0000x reference)
"""Multi-head attention (no mask) on 8 trn2 NeuronCores.

Problem: x[4,2048,1024] @ w_attn[1024,3072] + b_attn -> qkv, 16 heads x 64,
softmax(q k^T / 8) v, merge heads, @ w_proj[1024,1024] + b_proj.

Sharding: core c = (batch b = c//2, head-group g = c%2).  Each core handles
one batch and 8 heads (tensor-parallel over heads), producing a partial
c_proj output; the host adds the two partials per batch plus b_proj.

Device layout (all fp32):
  xT   [C, T]     host-pretransposed activation (c on partitions on chip)
  qT,kT[512, T]   = (x @ w_q/k + b)^T, stored as 4 chunks of [128, T]
                   (each chunk = 2 heads stacked 64+64 on partitions)
  v    [T, 512]   natural layout, 16 chunks of [128, 512]
  S^T  [j, i]     per head via row-tiled matmuls (k^T stationary)
  exp  on ACT with fused 1/8 scale, no max subtraction (scores are O(5))
  den  = sum_j exp via ones-matmuls (col-tiled into 32-row PSUM strips)
  y^T  [d, i]     via v-stationary col-tiled matmuls (2 heads concurrent)
  out  [T, 1024]  = y^T.T @ w_proj chunks, accumulated over 4 dcat chunks
"""

import numpy as np
from contextlib import ExitStack

import concourse.bass as bass
import concourse.tile as tile
from concourse import bacc, mybir
from concourse.bass_utils import run_bass_kernel_spmd

F32 = mybir.dt.float32
EXP = mybir.ActivationFunctionType.Exp

B, T_FULL, C = 4, 2048, 1024
N_HEAD, HEAD_DIM = 16, 64
HPG = 8           # heads per group (per core)
QKD = HPG * HEAD_DIM   # 512: per-core q/k/v width
N_CORES = 8
SCALE = 1.0 / np.sqrt(HEAD_DIM)

# test.py can flip these to get a profile out of the run
TRACE = False
LAST_RESULTS = None


def build_bass(T=T_FULL, exp_bf16=False):
    """Build the per-core Bass program (same program for all 8 cores)."""
    NCC = C // 128          # 8 c-chunks
    NTC = T // 128          # t-chunks (16 at full size)
    TH_SIZE = T // 2        # phase-1 t-half
    NI_TH = TH_SIZE // 512 if TH_SIZE >= 512 else 1   # 512-col mm splits
    I_BLK = min(512, T)
    N_I = T // I_BLK        # i-blocks (4 at full size)
    N_PAIR = HPG // 2       # 4 head pairs

    # Bacc (not raw Bass): its compile() runs generate_event_semaphores,
    # which legalizes multi-wait instructions (HW allows 1 wait/inst).
    nc = bacc.Bacc("TRN2", target_bir_lowering=False, debug=False,
                   num_devices=N_CORES)

    xT = nc.dram_tensor("xT", [C, T], F32, kind="ExternalInput").ap()
    w_qk = nc.dram_tensor("w_qk", [C, 2 * QKD], F32, kind="ExternalInput").ap()
    w_v = nc.dram_tensor("w_v", [C, QKD], F32, kind="ExternalInput").ap()
    b_qk = nc.dram_tensor("b_qk", [128, 8], F32, kind="ExternalInput").ap()
    b_v_bc = nc.dram_tensor("b_v_bc", [128, QKD], F32, kind="ExternalInput").ap()
    w_pr = nc.dram_tensor("w_pr", [QKD, C], F32, kind="ExternalInput").ap()
    ones = nc.dram_tensor("ones", [128, 64], F32, kind="ExternalInput").ap()
    out = nc.dram_tensor("out", [T, C], F32, kind="ExternalOutput").ap()

    edt = mybir.dt.bfloat16 if exp_bf16 else F32

    with tile.TileContext(nc) as tc, ExitStack() as ctx:
        persist = ctx.enter_context(tc.tile_pool(name="persist", bufs=1))
        qT = persist.tile([128, N_PAIR, T], F32)
        kT = persist.tile([128, N_PAIR, T], F32)
        v = persist.tile([128, NTC, QKD], edt)
        ones_sb = persist.tile([128, 64], edt)
        bqk_sb = persist.tile([128, 8], F32)
        bvbc_sb = persist.tile([128, QKD], F32)

        dvescr = persist.tile([1, 8], F32)
        nc.sync.dma_start(out=bqk_sb[:], in_=b_qk)
        nc.sync.dma_start(out=bvbc_sb[:], in_=b_v_bc)
        # DVE-side fences: TT/TS instructions also hold only one sync
        # wait, so absorb each bias-DMA wait into a tiny copy first
        nc.vector.tensor_copy(dvescr[0:1, 0:1], bqk_sb[0:1, 0:1])
        nc.vector.tensor_copy(dvescr[0:1, 1:2], bvbc_sb[0:1, 0:1])
        if exp_bf16:
            nc.gpsimd.dma_start(out=ones_sb[:], in_=ones)  # casts f32->bf16
            ones_f32 = persist.tile([128, 64], F32)
            nc.sync.dma_start(out=ones_f32[:], in_=ones)
        else:
            nc.sync.dma_start(out=ones_sb[:], in_=ones)
            ones_f32 = ones_sb

        # ---------------- phase 1: projections ----------------
        # fp32 matmuls lower to a fused weight-load whose ISA slot holds
        # only ONE sync wait; a matmul whose two inputs arrive on two
        # different DMA lanes gets two waits and fails walrus codegen.
        # "Fence" each freshly-DMA'd matmul input with a 1x1x1 dummy
        # matmul so real matmuls see at most one unobserved semaphore.
        with tc.tile_pool(name="ph1w", bufs=1) as ph1w, \
             tc.tile_pool(name="ph1x", bufs=1) as ph1x, \
             tc.tile_pool(name="pp_qk", bufs=2, space="PSUM") as pp_qk, \
             tc.tile_pool(name="pp_v", bufs=2, space="PSUM") as pp_v:
            fence_ps = pp_v.tile([1, 8], F32, tag="fence")
            nc.tensor.matmul(fence_ps[0:1, 0:1], ones_sb[0:1, 0:1],
                             ones_sb[0:1, 0:1], start=True, stop=True)
            if exp_bf16:
                nc.tensor.matmul(fence_ps[0:1, 1:2], ones_f32[0:1, 0:1],
                                 ones_f32[0:1, 0:1], start=True, stop=True)

            wqk_sb = ph1w.tile([128, NCC, 2 * QKD], F32)
            wv_sb = ph1w.tile([128, NCC, QKD], F32)
            nc.sync.dma_start(out=wqk_sb[:],
                              in_=w_qk.rearrange("(c p) n -> p c n", p=128))
            nc.sync.dma_start(out=wv_sb[:],
                              in_=w_v.rearrange("(c p) n -> p c n", p=128))
            nc.tensor.matmul(fence_ps[0:1, 2:3], wqk_sb[0:1, 0, 0:1],
                             ones_f32[0:1, 0:1], start=True, stop=True)
            nc.tensor.matmul(fence_ps[0:1, 3:4], wv_sb[0:1, 0, 0:1],
                             ones_f32[0:1, 0:1], start=True, stop=True)
            xT_r = xT.rearrange("(c p) t -> p c t", p=128)

            for th in range(2):
                tsl = slice(th * TH_SIZE, (th + 1) * TH_SIZE)
                xt_sb = ph1x.tile([128, NCC, TH_SIZE], F32, tag="xt")
                nc.sync.dma_start(out=xt_sb[:], in_=xT_r[:, :, tsl])
                nc.tensor.matmul(fence_ps[0:1, 4 + th:5 + th],
                                 xt_sb[0:1, 0, 0:1], ones_f32[0:1, 0:1],
                                 start=True, stop=True)

                # v natural: xT tiles stationary, w_v streaming.
                # (v before q/k so the DVE tick of the LAST phase-1
                # eviction the attention S-matmuls wait on already covers
                # every v eviction -> y-matmuls carry no extra DVE wait.)
                for tcl in range(TH_SIZE // 128):
                    tg = th * (TH_SIZE // 128) + tcl
                    psv = pp_v.tile([128, QKD], F32, tag="psv")
                    for c in range(NCC):
                        nc.tensor.matmul(
                            psv[:],
                            xt_sb[:, c, tcl * 128:(tcl + 1) * 128],
                            wv_sb[:, c, :],
                            start=(c == 0), stop=(c == NCC - 1))
                    nc.vector.tensor_add(v[:, tg, :], psv[:], bvbc_sb[:])

                # q^T and k^T: w chunks stationary, xT streaming
                for dc in range(8):
                    ps = pp_qk.tile([128, TH_SIZE], F32, tag="psqk")
                    for c in range(NCC):
                        for i2 in range(NI_TH):
                            isl = slice(i2 * 512, min((i2 + 1) * 512, TH_SIZE))
                            nc.tensor.matmul(
                                ps[:, isl],
                                wqk_sb[:, c, dc * 128:(dc + 1) * 128],
                                xt_sb[:, c, isl],
                                start=(c == 0), stop=(c == NCC - 1))
                    dst = (qT if dc < 4 else kT)[:, dc % 4, tsl]
                    nc.vector.tensor_scalar_add(dst, ps[:], bqk_sb[:, dc:dc + 1])

        # ---------------- phase 2: attention ----------------
        persist2 = ctx.enter_context(tc.tile_pool(name="persist2", bufs=1))
        yT = persist2.tile([128, N_PAIR, T], F32)
        with tc.tile_pool(name="att_s", bufs=2, space="PSUM") as s_pool, \
             tc.tile_pool(name="att_y", bufs=2, space="PSUM") as y_pool, \
             tc.tile_pool(name="att_d", bufs=2, space="PSUM") as d_pool, \
             tc.tile_pool(name="att_es", bufs=3) as es_pool, \
             tc.tile_pool(name="att_yr", bufs=4) as yr_pool, \
             tc.tile_pool(name="att_rc", bufs=8) as rc_pool, \
             tc.tile_pool(name="att_bc", bufs=2) as bc_pool:
            for p in range(N_PAIR):
                den_A = d_pool.tile([128, I_BLK], F32, tag="den")
                den_B = d_pool.tile([128, I_BLK], F32, tag="den")
                yraw = yr_pool.tile([128, T], F32, tag="yraw")
                for i in range(N_I):
                    isl = slice(i * I_BLK, (i + 1) * I_BLK)
                    # two banks: head A accumulates in yA[0:64], head B in
                    # yB[64:128] (start=True clears has_written bank-wide, so
                    # concurrent groups must not share a bank)
                    yA = y_pool.tile([128, I_BLK], F32, tag="y")
                    yB = y_pool.tile([128, I_BLK], F32, tag="y")
                    sA = i % 4
                    sB = (i + 2) % 4
                    for j in range(NTC):
                        jsl = slice(j * 128, (j + 1) * 128)
                        s = s_pool.tile([128, 2 * I_BLK], F32, tag="s")
                        nc.tensor.matmul(s[:, 0:I_BLK],
                                         kT[0:64, p, jsl], qT[0:64, p, isl],
                                         start=True, stop=True)
                        nc.tensor.matmul(s[:, I_BLK:2 * I_BLK],
                                         kT[64:128, p, jsl], qT[64:128, p, isl],
                                         start=True, stop=True)
                        es = es_pool.tile([128, 2 * I_BLK], edt, tag="es")
                        nc.scalar.activation(es[:], s[:], EXP, scale=SCALE)
                        nc.tensor.matmul(yA[0:64, :],
                                         v[:, j, p * 128:p * 128 + 64],
                                         es[:, 0:I_BLK],
                                         start=(j == 0), stop=(j == NTC - 1))
                        nc.tensor.matmul(yB[64:128, :],
                                         v[:, j, p * 128 + 64:p * 128 + 128],
                                         es[:, I_BLK:2 * I_BLK],
                                         start=(j == 0), stop=(j == NTC - 1))
                        nc.tensor.matmul(den_A[32 * sA:32 * sA + 32, :],
                                         ones_sb[:, 0:32], es[:, 0:I_BLK],
                                         start=(j == 0), stop=(j == NTC - 1),
                                         tile_position=(0, 32 * sA))
                        nc.tensor.matmul(den_B[32 * sB:32 * sB + 32, :],
                                         ones_sb[:, 0:32], es[:, I_BLK:2 * I_BLK],
                                         start=(j == 0), stop=(j == NTC - 1),
                                         tile_position=(0, 32 * sB))
                    nc.vector.tensor_copy(yraw[0:64, isl], yA[0:64, :])
                    nc.vector.tensor_copy(yraw[64:128, isl], yB[64:128, :])
                # reciprocals of the denominators (redundant rows are fine)
                rc_A = rc_pool.tile([128, I_BLK], F32, tag="rc")
                rc_B = rc_pool.tile([128, I_BLK], F32, tag="rc")
                # only strips actually written (matters when N_I < 4)
                rA = slice(0, 32 * min(N_I, 4))
                rB = slice(0, 128) if N_I >= 4 else slice(64, 64 + 32 * N_I)
                nc.vector.reciprocal(rc_A[rA, :], den_A[rA, :])
                nc.vector.reciprocal(rc_B[rB, :], den_B[rB, :])
                # carrier: bring PE's observed DVE tick up to the recips
                # so the bc matmuls (and next pair's den matmuls) add no
                # second wait. Writes a dead 1x1 into den_A (post-recip).
                nc.tensor.matmul(den_A[0:1, 0:1], rc_B[64:65, 0:1],
                                 ones_f32[64:65, 0:1], start=True, stop=True,
                                 tile_position=(64, 0))
                for i in range(N_I):
                    isl = slice(i * I_BLK, (i + 1) * I_BLK)
                    sA = i % 4
                    sB = (i + 2) % 4
                    bc = s_pool.tile([128, 2 * I_BLK], F32, tag="s")
                    nc.tensor.matmul(bc[0:64, 0:I_BLK],
                                     ones_f32[32 * sA:32 * sA + 1, 0:64],
                                     rc_A[32 * sA:32 * sA + 1, :],
                                     start=True, stop=True,
                                     tile_position=(32 * sA, 0))
                    nc.tensor.matmul(bc[64:128, 0:I_BLK],
                                     ones_f32[32 * sB:32 * sB + 1, 0:64],
                                     rc_B[32 * sB:32 * sB + 1, :],
                                     start=True, stop=True,
                                     tile_position=(32 * sB, 64))
                    bcs = bc_pool.tile([128, I_BLK], F32, tag="bcs")
                    nc.vector.tensor_copy(bcs[:], bc[:, 0:I_BLK])
                    nc.vector.tensor_mul(yT[:, p, isl], yraw[:, isl], bcs[:])

        # ---------------- phase 3: output projection ----------------
        with tc.tile_pool(name="ph3w", bufs=1) as ph3w, \
             tc.tile_pool(name="ph3o", bufs=3) as ph3o, \
             tc.tile_pool(name="pp_o", bufs=3, space="PSUM") as pp_o, \
             tc.tile_pool(name="pp_f3", bufs=1, space="PSUM") as pp_f3:
            wp_sb = ph3w.tile([128, N_PAIR, C], F32)
            nc.sync.dma_start(out=wp_sb[:],
                              in_=w_pr.rearrange("(d p) n -> p d n", p=128))
            f3 = pp_f3.tile([1, 8], F32, tag="fence3")
            nc.tensor.matmul(f3[0:1, 0:1], wp_sb[0:1, 0, 0:1],
                             ones_f32[0:1, 0:1], start=True, stop=True)
            for tcl in range(NTC):
                ps = pp_o.tile([128, C], F32, tag="pso")
                for d in range(N_PAIR):
                    for n2 in range(C // 512):
                        nsl = slice(n2 * 512, (n2 + 1) * 512)
                        nc.tensor.matmul(
                            ps[:, nsl],
                            yT[:, d, tcl * 128:(tcl + 1) * 128],
                            wp_sb[:, d, nsl],
                            start=(d == 0), stop=(d == N_PAIR - 1))
                os = ph3o.tile([128, C], F32, tag="os")
                # absorb the WAR wait on the slot's previous out-DMA
                nc.vector.memset(os[0:1, 0:1], 0.0)
                nc.vector.tensor_copy(os[:], ps[:])
                nc.sync.dma_start(out=out[tcl * 128:(tcl + 1) * 128, :], in_=os[:])

    nc.compile()
    return nc


def make_in_maps(x, w_attn, b_attn, w_proj, T=T_FULL):
    """Host-side sharding: per-core input dict."""
    x = np.asarray(x, dtype=np.float32)
    w_attn = np.asarray(w_attn, dtype=np.float32)
    b_attn = np.asarray(b_attn, dtype=np.float32)
    w_proj = np.asarray(w_proj, dtype=np.float32)
    in_maps = []
    ones = np.ones((128, 64), dtype=np.float32)
    for core in range(N_CORES):
        b, g = core // 2, core % 2
        gq = slice(g * QKD, (g + 1) * QKD)
        gk = slice(C + g * QKD, C + (g + 1) * QKD)
        gv = slice(2 * C + g * QKD, 2 * C + (g + 1) * QKD)
        w_qk = np.concatenate([w_attn[:, gq], w_attn[:, gk]], axis=1)
        b_q = b_attn[gq]
        b_k = b_attn[gk]
        b_v = b_attn[gv]
        b_qk = np.stack([b_q.reshape(4, 128), b_k.reshape(4, 128)],
                        axis=0).reshape(8, 128).T.copy()   # [128, 8]
        in_maps.append({
            "xT": np.ascontiguousarray(x[b, :T].T),
            "w_qk": np.ascontiguousarray(w_qk),
            "w_v": np.ascontiguousarray(w_attn[:, gv]),
            "b_qk": np.ascontiguousarray(b_qk),
            "b_v_bc": np.tile(b_v, (128, 1)),
            "w_pr": np.ascontiguousarray(w_proj[gq]),
            "ones": ones,
        })
    return in_maps


def kernel(x, w_attn, b_attn, w_proj, b_proj):
    global LAST_RESULTS
    in_maps = make_in_maps(x, w_attn, b_attn, w_proj)
    nc = build_bass()
    res = run_bass_kernel_spmd(nc, in_maps, list(range(N_CORES)), trace=TRACE)
    LAST_RESULTS = res
    b_proj = np.asarray(b_proj, dtype=np.float32)
    out = np.empty((B, T_FULL, C), dtype=np.float32)
    for b in range(B):
        out[b] = res.results[2 * b]["out"] + res.results[2 * b + 1]["out"] \
            + b_proj[None, :]
    return out


# revision 18
# speedup vs baseline: 1.7180x; 1.7180x over previous
"""Multi-head attention (no mask) on 8 trn2 NeuronCores.

Problem: x[4,2048,1024] @ w_attn[1024,3072] + b_attn -> qkv, 16 heads x 64,
softmax(q k^T / 8) v, merge heads, @ w_proj[1024,1024] + b_proj.

Sharding: core c = (batch b = c//2, head-group g = c%2).  Each core handles
one batch and 8 heads (tensor-parallel over heads), producing a partial
c_proj output; the host adds the two partials per batch plus b_proj.

Device layout (all fp32):
  xT   [C, T]     host-pretransposed activation (c on partitions on chip)
  qT,kT[512, T]   = (x @ w_q/k + b)^T, stored as 4 chunks of [128, T]
                   (each chunk = 2 heads stacked 64+64 on partitions)
  v    [T, 512]   natural layout, 16 chunks of [128, 512]
  S^T  [j, i]     per head via row-tiled matmuls (k^T stationary)
  exp  on ACT with fused 1/8 scale, no max subtraction (scores are O(5))
  den  = sum_j exp via ones-matmuls (col-tiled into 32-row PSUM strips)
  y^T  [d, i]     via v-stationary col-tiled matmuls (2 heads concurrent)
  out  [T, 1024]  = y^T.T @ w_proj chunks, accumulated over 4 dcat chunks
"""

import numpy as np
from contextlib import ExitStack

import concourse.bass as bass
import concourse.tile as tile
from concourse import bacc, mybir
from concourse.bass_utils import run_bass_kernel_spmd

F32 = mybir.dt.float32
EXP = mybir.ActivationFunctionType.Exp

B, T_FULL, C = 4, 2048, 1024
N_HEAD, HEAD_DIM = 16, 64
HPG = 8           # heads per group (per core)
QKD = HPG * HEAD_DIM   # 512: per-core q/k/v width
N_CORES = 8
SCALE = 1.0 / np.sqrt(HEAD_DIM)

# test.py can flip these to get a profile out of the run
TRACE = False
LAST_RESULTS = None


def build_bass(T=T_FULL, use_bf16=True):
    """Build the per-core Bass program (same program for all 8 cores)."""
    NCC = C // 128          # 8 c-chunks
    NTC = T // 128          # t-chunks (16 at full size)
    TH_SIZE = T // 2        # phase-1 t-half
    NI_TH = TH_SIZE // 512 if TH_SIZE >= 512 else 1   # 512-col mm splits
    I_BLK = min(512, T)
    N_I = T // I_BLK        # i-blocks (4 at full size)
    N_PAIR = HPG // 2       # 4 head pairs

    # Bacc (not raw Bass): its compile() runs generate_event_semaphores,
    # which legalizes multi-wait instructions (HW allows 1 wait/inst).
    nc = bacc.Bacc("TRN2", target_bir_lowering=False, debug=False,
                   num_devices=N_CORES)

    mdt = mybir.dt.bfloat16 if use_bf16 else F32
    xT = nc.dram_tensor("xT", [C, T], mdt, kind="ExternalInput").ap()
    w_qk = nc.dram_tensor("w_qk", [C, 2 * QKD], mdt, kind="ExternalInput").ap()
    w_v = nc.dram_tensor("w_v", [C, QKD], mdt, kind="ExternalInput").ap()
    b_qk = nc.dram_tensor("b_qk", [128, 8], F32, kind="ExternalInput").ap()
    b_v_bc = nc.dram_tensor("b_v_bc", [128, QKD], F32, kind="ExternalInput").ap()
    w_pr = nc.dram_tensor("w_pr", [QKD, C], mdt, kind="ExternalInput").ap()
    ones = nc.dram_tensor("ones", [128, 64], F32, kind="ExternalInput").ap()
    out = nc.dram_tensor("out", [T, C], F32, kind="ExternalOutput").ap()

    BF = mybir.dt.bfloat16
    edt = BF if use_bf16 else F32

    with tile.TileContext(nc) as tc, ExitStack() as ctx:
        persist = ctx.enter_context(tc.tile_pool(name="persist", bufs=1))
        qT = persist.tile([128, N_PAIR, T], edt)
        kT = persist.tile([128, N_PAIR, T], edt)
        v = persist.tile([128, NTC, QKD], edt)
        ones_sb = persist.tile([128, 64], edt)
        bqk_sb = persist.tile([128, 8], F32)
        bvbc_sb = persist.tile([128, QKD], F32)

        dvescr = persist.tile([1, 8], F32)
        nc.sync.dma_start(out=bqk_sb[:], in_=b_qk)
        nc.sync.dma_start(out=bvbc_sb[:], in_=b_v_bc)
        # DVE-side fences: TT/TS instructions also hold only one sync
        # wait, so absorb each bias-DMA wait into a tiny copy first
        nc.vector.tensor_copy(dvescr[0:1, 0:1], bqk_sb[0:1, 0:1])
        nc.vector.tensor_copy(dvescr[0:1, 1:2], bvbc_sb[0:1, 0:1])
        if use_bf16:
            nc.gpsimd.dma_start(out=ones_sb[:], in_=ones)  # casts f32->bf16
            ones_f32 = persist.tile([128, 64], F32)
            nc.sync.dma_start(out=ones_f32[:], in_=ones)
        else:
            nc.sync.dma_start(out=ones_sb[:], in_=ones)
            ones_f32 = ones_sb

        # ---------------- phase 1: projections ----------------
        # fp32 matmuls lower to a fused weight-load whose ISA slot holds
        # only ONE sync wait; a matmul whose two inputs arrive on two
        # different DMA lanes gets two waits and fails walrus codegen.
        # "Fence" each freshly-DMA'd matmul input with a 1x1x1 dummy
        # matmul so real matmuls see at most one unobserved semaphore.
        with tc.tile_pool(name="ph1w", bufs=1) as ph1w, \
             tc.tile_pool(name="ph1x", bufs=1) as ph1x, \
             tc.tile_pool(name="pp_qk", bufs=2, space="PSUM") as pp_qk, \
             tc.tile_pool(name="pp_v", bufs=2, space="PSUM") as pp_v:
            fence_ps = pp_v.tile([1, 8], F32, tag="fence")
            nc.tensor.matmul(fence_ps[0:1, 0:1], ones_sb[0:1, 0:1],
                             ones_sb[0:1, 0:1], start=True, stop=True)
            if use_bf16:
                nc.tensor.matmul(fence_ps[0:1, 1:2], ones_f32[0:1, 0:1],
                                 ones_f32[0:1, 0:1], start=True, stop=True)

            wqk_sb = ph1w.tile([128, NCC, 2 * QKD], edt)
            wv_sb = ph1w.tile([128, NCC, QKD], edt)
            nc.sync.dma_start(out=wqk_sb[:],
                              in_=w_qk.rearrange("(c p) n -> p c n", p=128))
            nc.sync.dma_start(out=wv_sb[:],
                              in_=w_v.rearrange("(c p) n -> p c n", p=128))
            nc.tensor.matmul(fence_ps[0:1, 2:3], wqk_sb[0:1, 0, 0:1],
                             ones_sb[0:1, 0:1], start=True, stop=True)
            nc.tensor.matmul(fence_ps[0:1, 3:4], wv_sb[0:1, 0, 0:1],
                             ones_sb[0:1, 0:1], start=True, stop=True)
            xT_r = xT.rearrange("(c p) t -> p c t", p=128)

            for th in range(2):
                tsl = slice(th * TH_SIZE, (th + 1) * TH_SIZE)
                xt_sb = ph1x.tile([128, NCC, TH_SIZE], edt, tag="xt")
                nc.sync.dma_start(out=xt_sb[:], in_=xT_r[:, :, tsl])
                nc.tensor.matmul(fence_ps[0:1, 4 + th:5 + th],
                                 xt_sb[0:1, 0, 0:1], ones_sb[0:1, 0:1],
                                 start=True, stop=True)

                # v natural: xT tiles stationary, w_v streaming.
                # (v before q/k so the DVE tick of the LAST phase-1
                # eviction the attention S-matmuls wait on already covers
                # every v eviction -> y-matmuls carry no extra DVE wait.)
                for tcl in range(TH_SIZE // 128):
                    tg = th * (TH_SIZE // 128) + tcl
                    psv = pp_v.tile([128, QKD], F32, tag="psv")
                    for c in range(NCC):
                        nc.tensor.matmul(
                            psv[:],
                            xt_sb[:, c, tcl * 128:(tcl + 1) * 128],
                            wv_sb[:, c, :],
                            start=(c == 0), stop=(c == NCC - 1))
                    nc.vector.tensor_add(v[:, tg, :], psv[:], bvbc_sb[:])

                # q^T and k^T: w chunks stationary, xT streaming
                for dc in range(8):
                    ps = pp_qk.tile([128, TH_SIZE], F32, tag="psqk")
                    for c in range(NCC):
                        for i2 in range(NI_TH):
                            isl = slice(i2 * 512, min((i2 + 1) * 512, TH_SIZE))
                            nc.tensor.matmul(
                                ps[:, isl],
                                wqk_sb[:, c, dc * 128:(dc + 1) * 128],
                                xt_sb[:, c, isl],
                                start=(c == 0), stop=(c == NCC - 1))
                    dst = (qT if dc < 4 else kT)[:, dc % 4, tsl]
                    nc.vector.tensor_scalar_add(dst, ps[:], bqk_sb[:, dc:dc + 1])

        # ---------------- phase 2: attention ----------------
        persist2 = ctx.enter_context(tc.tile_pool(name="persist2", bufs=1))
        yT = persist2.tile([128, N_PAIR, T], edt)
        with tc.tile_pool(name="att_s", bufs=2, space="PSUM") as s_pool, \
             tc.tile_pool(name="att_y", bufs=2, space="PSUM") as y_pool, \
             tc.tile_pool(name="att_d", bufs=2, space="PSUM") as d_pool, \
             tc.tile_pool(name="att_es", bufs=3) as es_pool, \
             tc.tile_pool(name="att_yr", bufs=4) as yr_pool, \
             tc.tile_pool(name="att_rc", bufs=8) as rc_pool, \
             tc.tile_pool(name="att_bc", bufs=2) as bc_pool:
            for p in range(N_PAIR):
                den_A = d_pool.tile([128, I_BLK], F32, tag="den")
                den_B = d_pool.tile([128, I_BLK], F32, tag="den")
                yraw = yr_pool.tile([128, T], F32, tag="yraw")
                for i in range(N_I):
                    isl = slice(i * I_BLK, (i + 1) * I_BLK)
                    # two banks: head A accumulates in yA[0:64], head B in
                    # yB[64:128] (start=True clears has_written bank-wide, so
                    # concurrent groups must not share a bank)
                    yA = y_pool.tile([128, I_BLK], F32, tag="y")
                    yB = y_pool.tile([128, I_BLK], F32, tag="y")
                    sA = i % 4
                    sB = (i + 2) % 4
                    for j in range(NTC):
                        jsl = slice(j * 128, (j + 1) * 128)
                        s = s_pool.tile([128, 2 * I_BLK], F32, tag="s")
                        nc.tensor.matmul(s[:, 0:I_BLK],
                                         kT[0:64, p, jsl], qT[0:64, p, isl],
                                         start=True, stop=True)
                        nc.tensor.matmul(s[:, I_BLK:2 * I_BLK],
                                         kT[64:128, p, jsl], qT[64:128, p, isl],
                                         start=True, stop=True)
                        es = es_pool.tile([128, 2 * I_BLK], edt, tag="es")
                        nc.scalar.activation(es[:], s[:], EXP, scale=SCALE)
                        nc.tensor.matmul(yA[0:64, :],
                                         v[:, j, p * 128:p * 128 + 64],
                                         es[:, 0:I_BLK],
                                         start=(j == 0), stop=(j == NTC - 1))
                        nc.tensor.matmul(yB[64:128, :],
                                         v[:, j, p * 128 + 64:p * 128 + 128],
                                         es[:, I_BLK:2 * I_BLK],
                                         start=(j == 0), stop=(j == NTC - 1))
                        nc.tensor.matmul(den_A[32 * sA:32 * sA + 32, :],
                                         ones_sb[:, 0:32], es[:, 0:I_BLK],
                                         start=(j == 0), stop=(j == NTC - 1),
                                         tile_position=(0, 32 * sA))
                        nc.tensor.matmul(den_B[32 * sB:32 * sB + 32, :],
                                         ones_sb[:, 0:32], es[:, I_BLK:2 * I_BLK],
                                         start=(j == 0), stop=(j == NTC - 1),
                                         tile_position=(0, 32 * sB))
                    nc.vector.tensor_copy(yraw[0:64, isl], yA[0:64, :])
                    nc.vector.tensor_copy(yraw[64:128, isl], yB[64:128, :])
                # reciprocals of the denominators (redundant rows are fine)
                rc_A = rc_pool.tile([128, I_BLK], F32, tag="rc")
                rc_B = rc_pool.tile([128, I_BLK], F32, tag="rc")
                # only strips actually written (matters when N_I < 4)
                rA = slice(0, 32 * min(N_I, 4))
                rB = slice(0, 128) if N_I >= 4 else slice(64, 64 + 32 * N_I)
                nc.vector.reciprocal(rc_A[rA, :], den_A[rA, :])
                nc.vector.reciprocal(rc_B[rB, :], den_B[rB, :])
                # carrier: bring PE's observed DVE tick up to the recips
                # so the bc matmuls (and next pair's den matmuls) add no
                # second wait. Writes a dead 1x1 into den_A (post-recip).
                nc.tensor.matmul(den_A[0:1, 0:1], rc_B[64:65, 0:1],
                                 ones_f32[64:65, 0:1], start=True, stop=True,
                                 tile_position=(64, 0))
                for i in range(N_I):
                    isl = slice(i * I_BLK, (i + 1) * I_BLK)
                    sA = i % 4
                    sB = (i + 2) % 4
                    bc = s_pool.tile([128, 2 * I_BLK], F32, tag="s")
                    nc.tensor.matmul(bc[0:64, 0:I_BLK],
                                     ones_f32[32 * sA:32 * sA + 1, 0:64],
                                     rc_A[32 * sA:32 * sA + 1, :],
                                     start=True, stop=True,
                                     tile_position=(32 * sA, 0))
                    nc.tensor.matmul(bc[64:128, 0:I_BLK],
                                     ones_f32[32 * sB:32 * sB + 1, 0:64],
                                     rc_B[32 * sB:32 * sB + 1, :],
                                     start=True, stop=True,
                                     tile_position=(32 * sB, 64))
                    bcs = bc_pool.tile([128, I_BLK], F32, tag="bcs")
                    nc.vector.tensor_copy(bcs[:], bc[:, 0:I_BLK])
                    nc.vector.tensor_mul(yT[:, p, isl], yraw[:, isl], bcs[:])

        # ---------------- phase 3: output projection ----------------
        with tc.tile_pool(name="ph3w", bufs=1) as ph3w, \
             tc.tile_pool(name="ph3o", bufs=3) as ph3o, \
             tc.tile_pool(name="pp_o", bufs=3, space="PSUM") as pp_o, \
             tc.tile_pool(name="pp_f3", bufs=1, space="PSUM") as pp_f3:
            wp_sb = ph3w.tile([128, N_PAIR, C], edt)
            nc.sync.dma_start(out=wp_sb[:],
                              in_=w_pr.rearrange("(d p) n -> p d n", p=128))
            f3 = pp_f3.tile([1, 8], F32, tag="fence3")
            nc.tensor.matmul(f3[0:1, 0:1], wp_sb[0:1, 0, 0:1],
                             ones_sb[0:1, 0:1], start=True, stop=True)
            for tcl in range(NTC):
                ps = pp_o.tile([128, C], F32, tag="pso")
                for d in range(N_PAIR):
                    for n2 in range(C // 512):
                        nsl = slice(n2 * 512, (n2 + 1) * 512)
                        nc.tensor.matmul(
                            ps[:, nsl],
                            yT[:, d, tcl * 128:(tcl + 1) * 128],
                            wp_sb[:, d, nsl],
                            start=(d == 0), stop=(d == N_PAIR - 1))
                os = ph3o.tile([128, C], F32, tag="os")
                # absorb the WAR wait on the slot's previous out-DMA
                nc.vector.memset(os[0:1, 0:1], 0.0)
                nc.vector.tensor_copy(os[:], ps[:])
                nc.sync.dma_start(out=out[tcl * 128:(tcl + 1) * 128, :], in_=os[:])

    nc.compile()
    return nc


def make_in_maps(x, w_attn, b_attn, w_proj, T=T_FULL, use_bf16=True):
    """Host-side sharding: per-core input dict."""
    import ml_dtypes
    mdt = ml_dtypes.bfloat16 if use_bf16 else np.float32
    x = np.asarray(x, dtype=np.float32)
    w_attn = np.asarray(w_attn, dtype=np.float32)
    b_attn = np.asarray(b_attn, dtype=np.float32)
    w_proj = np.asarray(w_proj, dtype=np.float32)
    in_maps = []
    ones = np.ones((128, 64), dtype=np.float32)
    for core in range(N_CORES):
        b, g = core // 2, core % 2
        gq = slice(g * QKD, (g + 1) * QKD)
        gk = slice(C + g * QKD, C + (g + 1) * QKD)
        gv = slice(2 * C + g * QKD, 2 * C + (g + 1) * QKD)
        w_qk = np.concatenate([w_attn[:, gq], w_attn[:, gk]], axis=1)
        b_q = b_attn[gq]
        b_k = b_attn[gk]
        b_v = b_attn[gv]
        b_qk = np.stack([b_q.reshape(4, 128), b_k.reshape(4, 128)],
                        axis=0).reshape(8, 128).T.copy()   # [128, 8]
        in_maps.append({
            "xT": np.ascontiguousarray(x[b, :T].T).astype(mdt),
            "w_qk": np.ascontiguousarray(w_qk).astype(mdt),
            "w_v": np.ascontiguousarray(w_attn[:, gv]).astype(mdt),
            "b_qk": np.ascontiguousarray(b_qk),
            "b_v_bc": np.tile(b_v, (128, 1)),
            "w_pr": np.ascontiguousarray(w_proj[gq]).astype(mdt),
            "ones": ones,
        })
    return in_maps


def kernel(x, w_attn, b_attn, w_proj, b_proj):
    global LAST_RESULTS
    in_maps = make_in_maps(x, w_attn, b_attn, w_proj)
    nc = build_bass()
    res = run_bass_kernel_spmd(nc, in_maps, list(range(N_CORES)), trace=TRACE)
    LAST_RESULTS = res
    b_proj = np.asarray(b_proj, dtype=np.float32)
    out = np.empty((B, T_FULL, C), dtype=np.float32)
    for b in range(B):
        out[b] = res.results[2 * b]["out"] + res.results[2 * b + 1]["out"] \
            + b_proj[None, :]
    return out


# revision 23
# speedup vs baseline: 2.9133x; 1.6958x over previous
"""Multi-head attention (no mask) on 8 trn2 NeuronCores.

Problem: x[4,2048,1024] @ w_attn[1024,3072] + b_attn -> qkv, 16 heads x 64,
softmax(q k^T / 8) v, merge heads, @ w_proj[1024,1024] + b_proj.

Sharding: core c = (batch b = c//2, head-group g = c%2).  Each core handles
one batch and 8 heads (tensor-parallel over heads), producing a partial
c_proj output; the host adds the two partials per batch plus b_proj.

Device layout (all fp32):
  xT   [C, T]     host-pretransposed activation (c on partitions on chip)
  qT,kT[512, T]   = (x @ w_q/k + b)^T, stored as 4 chunks of [128, T]
                   (each chunk = 2 heads stacked 64+64 on partitions)
  v    [T, 512]   natural layout, 16 chunks of [128, 512]
  S^T  [j, i]     per head via row-tiled matmuls (k^T stationary)
  exp  on ACT with fused 1/8 scale, no max subtraction (scores are O(5))
  den  = sum_j exp via ones-matmuls (col-tiled into 32-row PSUM strips)
  y^T  [d, i]     via v-stationary col-tiled matmuls (2 heads concurrent)
  out  [T, 1024]  = y^T.T @ w_proj chunks, accumulated over 4 dcat chunks
"""

import numpy as np
from contextlib import ExitStack

import concourse.bass as bass
import concourse.tile as tile
from concourse import bacc, mybir
from concourse.bass_utils import run_bass_kernel_spmd

F32 = mybir.dt.float32
EXP = mybir.ActivationFunctionType.Exp

B, T_FULL, C = 4, 2048, 1024
N_HEAD, HEAD_DIM = 16, 64
HPG = 8           # heads per group (per core)
QKD = HPG * HEAD_DIM   # 512: per-core q/k/v width
N_CORES = 8
SCALE = 1.0 / np.sqrt(HEAD_DIM)

# test.py can flip these to get a profile out of the run
TRACE = False
LAST_RESULTS = None


def build_bass(T=T_FULL, use_bf16=True):
    """Build the per-core Bass program (same program for all 8 cores)."""
    NCC = C // 128          # 8 c-chunks
    NTC = T // 128          # t-chunks (16 at full size)
    TH_SIZE = T // 2        # phase-1 t-half
    NI_TH = TH_SIZE // 512 if TH_SIZE >= 512 else 1   # 512-col mm splits
    I_BLK = min(512, T)
    N_I = T // I_BLK        # i-blocks (4 at full size)
    N_PAIR = HPG // 2       # 4 head pairs

    # Bacc (not raw Bass): its compile() runs generate_event_semaphores,
    # which legalizes multi-wait instructions (HW allows 1 wait/inst).
    nc = bacc.Bacc("TRN2", target_bir_lowering=False, debug=False,
                   num_devices=N_CORES)

    mdt = mybir.dt.bfloat16 if use_bf16 else F32
    xT = nc.dram_tensor("xT", [C, T], mdt, kind="ExternalInput").ap()
    w_qk = nc.dram_tensor("w_qk", [C, 2 * QKD], mdt, kind="ExternalInput").ap()
    w_v = nc.dram_tensor("w_v", [C, QKD], mdt, kind="ExternalInput").ap()
    b_qk = nc.dram_tensor("b_qk", [128, 8], F32, kind="ExternalInput").ap()
    b_v_bc = nc.dram_tensor("b_v_bc", [128, QKD], F32, kind="ExternalInput").ap()
    w_pr = nc.dram_tensor("w_pr", [QKD, C], mdt, kind="ExternalInput").ap()
    ones = nc.dram_tensor("ones", [128, 64], F32, kind="ExternalInput").ap()
    out = nc.dram_tensor("out", [T, C], F32, kind="ExternalOutput").ap()
    # DRAM bounce for softmax reciprocals (SBUF sources cannot
    # partition-broadcast, DRAM sources can)
    N_I_ = T // min(512, T)
    rcd = nc.dram_tensor("rc_scratch", [HPG // 2, 2 * N_I_, min(512, T)],
                         F32).ap()

    BF = mybir.dt.bfloat16
    edt = BF if use_bf16 else F32

    with tile.TileContext(nc) as tc, ExitStack() as ctx:
        persist = ctx.enter_context(tc.tile_pool(name="persist", bufs=1))
        qT = persist.tile([128, N_PAIR, T], edt)
        kT = persist.tile([128, N_PAIR, T], edt)
        # v stored 65-wide per head: 64 data cols + a ones column that
        # makes row 64 of each y matmul the softmax denominator
        v = persist.tile([128, NTC, HPG * 65], edt)
        ones_sb = persist.tile([128, 64], edt)
        bqk_sb = persist.tile([128, 8], F32)
        bvbc_sb = persist.tile([128, QKD], F32)

        dvescr = persist.tile([1, 8], F32)
        nc.sync.dma_start(out=bqk_sb[:], in_=b_qk)
        nc.sync.dma_start(out=bvbc_sb[:], in_=b_v_bc)
        # DVE-side fences: TT/TS instructions also hold only one sync
        # wait, so absorb each bias-DMA wait into a tiny copy first
        nc.vector.tensor_copy(dvescr[0:1, 0:1], bqk_sb[0:1, 0:1])
        nc.vector.tensor_copy(dvescr[0:1, 1:2], bvbc_sb[0:1, 0:1])
        if use_bf16:
            nc.gpsimd.dma_start(out=ones_sb[:], in_=ones)  # casts f32->bf16
            ones_f32 = persist.tile([128, 64], F32)
            nc.sync.dma_start(out=ones_f32[:], in_=ones)
        else:
            nc.sync.dma_start(out=ones_sb[:], in_=ones)
            ones_f32 = ones_sb

        # ---------------- phase 1: projections ----------------
        # fp32 matmuls lower to a fused weight-load whose ISA slot holds
        # only ONE sync wait; a matmul whose two inputs arrive on two
        # different DMA lanes gets two waits and fails walrus codegen.
        # "Fence" each freshly-DMA'd matmul input with a 1x1x1 dummy
        # matmul so real matmuls see at most one unobserved semaphore.
        with tc.tile_pool(name="ph1w", bufs=1) as ph1w, \
             tc.tile_pool(name="ph1x", bufs=1) as ph1x, \
             tc.tile_pool(name="pp_qk", bufs=2, space="PSUM") as pp_qk, \
             tc.tile_pool(name="pp_v", bufs=2, space="PSUM") as pp_v:
            fence_ps = pp_v.tile([1, 8], F32, tag="fence")
            nc.tensor.matmul(fence_ps[0:1, 0:1], ones_sb[0:1, 0:1],
                             ones_sb[0:1, 0:1], start=True, stop=True)
            if use_bf16:
                nc.tensor.matmul(fence_ps[0:1, 1:2], ones_f32[0:1, 0:1],
                                 ones_f32[0:1, 0:1], start=True, stop=True)

            wqk_sb = ph1w.tile([128, NCC, 2 * QKD], edt)
            wv_sb = ph1w.tile([128, NCC, QKD], edt)
            nc.sync.dma_start(out=wqk_sb[:],
                              in_=w_qk.rearrange("(c p) n -> p c n", p=128))
            nc.sync.dma_start(out=wv_sb[:],
                              in_=w_v.rearrange("(c p) n -> p c n", p=128))
            v_r = v[:, :, :].rearrange("q t (h e) -> q t h e", e=65)
            nc.vector.memset(v_r[:, :, :, 64:65], 1.0)
            nc.tensor.matmul(fence_ps[0:1, 2:3], wqk_sb[0:1, 0, 0:1],
                             ones_sb[0:1, 0:1], start=True, stop=True)
            nc.tensor.matmul(fence_ps[0:1, 3:4], wv_sb[0:1, 0, 0:1],
                             ones_sb[0:1, 0:1], start=True, stop=True)
            xT_r = xT.rearrange("(c p) t -> p c t", p=128)

            for th in range(2):
                tsl = slice(th * TH_SIZE, (th + 1) * TH_SIZE)
                xt_sb = ph1x.tile([128, NCC, TH_SIZE], edt, tag="xt")
                nc.sync.dma_start(out=xt_sb[:], in_=xT_r[:, :, tsl])
                nc.tensor.matmul(fence_ps[0:1, 4 + th:5 + th],
                                 xt_sb[0:1, 0, 0:1], ones_sb[0:1, 0:1],
                                 start=True, stop=True)

                # v natural: xT tiles stationary, w_v streaming.
                # (v before q/k so the DVE tick of the LAST phase-1
                # eviction the attention S-matmuls wait on already covers
                # every v eviction -> y-matmuls carry no extra DVE wait.)
                for tcl in range(TH_SIZE // 128):
                    tg = th * (TH_SIZE // 128) + tcl
                    psv = pp_v.tile([128, QKD], F32, tag="psv")
                    for c in range(NCC):
                        nc.tensor.matmul(
                            psv[:],
                            xt_sb[:, c, tcl * 128:(tcl + 1) * 128],
                            wv_sb[:, c, :],
                            start=(c == 0), stop=(c == NCC - 1))
                    nc.vector.tensor_add(
                        v[:, tg, :].rearrange("q (h e) -> q h e", e=65)[:, :, 0:64],
                        psv[:].rearrange("q (h e) -> q h e", e=64),
                        bvbc_sb[:].rearrange("q (h e) -> q h e", e=64))

                # q^T and k^T: w chunks stationary, xT streaming
                for dc in range(8):
                    ps = pp_qk.tile([128, TH_SIZE], F32, tag="psqk")
                    for c in range(NCC):
                        for i2 in range(NI_TH):
                            isl = slice(i2 * 512, min((i2 + 1) * 512, TH_SIZE))
                            nc.tensor.matmul(
                                ps[:, isl],
                                wqk_sb[:, c, dc * 128:(dc + 1) * 128],
                                xt_sb[:, c, isl],
                                start=(c == 0), stop=(c == NCC - 1))
                    dst = (qT if dc < 4 else kT)[:, dc % 4, tsl]
                    nc.vector.tensor_scalar_add(dst, ps[:], bqk_sb[:, dc:dc + 1])

        # ---------------- phase 2: attention ----------------
        # Per head: S^T via K=64 matmuls (row-pair per es grain), then
        # y accumulation with M=65 single-tile matmuls whose 65th lhsT
        # column is all-ones -> row 64 of the y accumulator is the
        # softmax denominator (free: matmul time is N-bound).
        persist2 = ctx.enter_context(tc.tile_pool(name="persist2", bufs=1))
        yT = persist2.tile([128, N_PAIR, T], edt)
        with tc.tile_pool(name="att_s", bufs=2, space="PSUM") as s_pool, \
             tc.tile_pool(name="att_y", bufs=2, space="PSUM") as y_pool, \
             tc.tile_pool(name="att_es", bufs=3) as es_pool, \
             tc.tile_pool(name="att_yr", bufs=4) as yr_pool, \
             tc.tile_pool(name="att_st", bufs=4) as st_pool, \
             tc.tile_pool(name="att_cl", bufs=2) as cl_pool, \
             tc.tile_pool(name="att_rc", bufs=2) as rc_pool, \
             tc.tile_pool(name="att_bc", bufs=4) as bc_pool:
            for p in range(N_PAIR):
                yraw_a = yr_pool.tile([64, T], edt, tag="yraw")
                yraw_b = yr_pool.tile([64, T], edt, tag="yraw")
                yraws = [yraw_a, yraw_b]
                coll = cl_pool.tile([2 * N_I, I_BLK], F32, tag="coll")
                for i in range(N_I):
                    isl = slice(i * I_BLK, (i + 1) * I_BLK)
                    y_a = y_pool.tile([65, I_BLK], F32, tag="y")
                    y_b = y_pool.tile([65, I_BLK], F32, tag="y")
                    ys = [y_a, y_b]
                    for j in range(NTC):
                        jsl = slice(j * 128, (j + 1) * 128)
                        s = s_pool.tile([128, 2 * I_BLK], F32, tag="s")
                        nc.tensor.matmul(s[:, 0:I_BLK],
                                         kT[0:64, p, jsl], qT[0:64, p, isl],
                                         start=True, stop=True)
                        nc.tensor.matmul(s[:, I_BLK:2 * I_BLK],
                                         kT[64:128, p, jsl], qT[64:128, p, isl],
                                         start=True, stop=True)
                        es = es_pool.tile([128, 2 * I_BLK], edt, tag="es")
                        nc.scalar.activation(es[:], s[:], EXP, scale=SCALE)
                        for hl in range(2):
                            h = 2 * p + hl
                            nc.tensor.matmul(
                                ys[hl][0:65, :],
                                v[:, j, 65 * h:65 * h + 65],
                                es[:, hl * I_BLK:(hl + 1) * I_BLK],
                                start=(j == 0), stop=(j == NTC - 1))
                    for hl in range(2):
                        st = st_pool.tile([65, I_BLK], F32, tag="st")
                        nc.vector.tensor_copy(st[64:65, :], ys[hl][64:65, :])
                        nc.vector.tensor_copy(yraws[hl][:, isl],
                                              ys[hl][0:64, :])
                        # move the denominator row to its own partition
                        r = hl * N_I + i
                        nc.gpsimd.dma_start(out=coll[r:r + 1, :],
                                            in_=st[64:65, :])
                rc = rc_pool.tile([2 * N_I, I_BLK], F32, tag="rc")
                nc.vector.reciprocal(rc[:], coll[:])
                nc.gpsimd.dma_start(out=rcd[p], in_=rc[:])
                for hl in range(2):
                    for i in range(N_I):
                        isl = slice(i * I_BLK, (i + 1) * I_BLK)
                        r = hl * N_I + i
                        bcast = bc_pool.tile([64, I_BLK], edt, tag="bcast")
                        rrow = rcd[p, r, :]
                        rbc = bass.AP(tensor=rrow.tensor, offset=rrow.offset,
                                      ap=[[0, 64]] + list(rrow.ap))
                        nc.gpsimd.dma_start(out=bcast[:], in_=rbc)
                        if hl == 0:
                            nc.vector.tensor_mul(yT[0:64, p, isl],
                                                 yraws[0][:, isl], bcast[:])
                        else:
                            ybst = bc_pool.tile([64, I_BLK], edt, tag="ybst")
                            nc.vector.tensor_mul(ybst[:], yraws[1][:, isl],
                                                 bcast[:])
                            nc.gpsimd.dma_start(out=yT[64:128, p, isl],
                                                in_=ybst[:])

        # ---------------- phase 3: output projection ----------------
        with tc.tile_pool(name="ph3w", bufs=1) as ph3w, \
             tc.tile_pool(name="ph3o", bufs=3) as ph3o, \
             tc.tile_pool(name="pp_o", bufs=3, space="PSUM") as pp_o, \
             tc.tile_pool(name="pp_f3", bufs=1, space="PSUM") as pp_f3:
            wp_sb = ph3w.tile([128, N_PAIR, C], edt)
            nc.sync.dma_start(out=wp_sb[:],
                              in_=w_pr.rearrange("(d p) n -> p d n", p=128))
            f3 = pp_f3.tile([1, 8], F32, tag="fence3")
            nc.tensor.matmul(f3[0:1, 0:1], wp_sb[0:1, 0, 0:1],
                             ones_sb[0:1, 0:1], start=True, stop=True)
            for tcl in range(NTC):
                ps = pp_o.tile([128, C], F32, tag="pso")
                for d in range(N_PAIR):
                    for n2 in range(C // 512):
                        nsl = slice(n2 * 512, (n2 + 1) * 512)
                        nc.tensor.matmul(
                            ps[:, nsl],
                            yT[:, d, tcl * 128:(tcl + 1) * 128],
                            wp_sb[:, d, nsl],
                            start=(d == 0), stop=(d == N_PAIR - 1))
                os = ph3o.tile([128, C], F32, tag="os")
                # absorb the WAR wait on the slot's previous out-DMA
                nc.vector.memset(os[0:1, 0:1], 0.0)
                nc.vector.tensor_copy(os[:], ps[:])
                nc.sync.dma_start(out=out[tcl * 128:(tcl + 1) * 128, :], in_=os[:])

    nc.compile()
    return nc


def make_in_maps(x, w_attn, b_attn, w_proj, T=T_FULL, use_bf16=True):
    """Host-side sharding: per-core input dict."""
    import ml_dtypes
    mdt = ml_dtypes.bfloat16 if use_bf16 else np.float32
    x = np.asarray(x, dtype=np.float32)
    w_attn = np.asarray(w_attn, dtype=np.float32)
    b_attn = np.asarray(b_attn, dtype=np.float32)
    w_proj = np.asarray(w_proj, dtype=np.float32)
    in_maps = []
    ones = np.ones((128, 64), dtype=np.float32)
    for core in range(N_CORES):
        b, g = core // 2, core % 2
        gq = slice(g * QKD, (g + 1) * QKD)
        gk = slice(C + g * QKD, C + (g + 1) * QKD)
        gv = slice(2 * C + g * QKD, 2 * C + (g + 1) * QKD)
        w_qk = np.concatenate([w_attn[:, gq], w_attn[:, gk]], axis=1)
        b_q = b_attn[gq]
        b_k = b_attn[gk]
        b_v = b_attn[gv]
        b_qk = np.stack([b_q.reshape(4, 128), b_k.reshape(4, 128)],
                        axis=0).reshape(8, 128).T.copy()   # [128, 8]
        in_maps.append({
            "xT": np.ascontiguousarray(x[b, :T].T).astype(mdt),
            "w_qk": np.ascontiguousarray(w_qk).astype(mdt),
            "w_v": np.ascontiguousarray(w_attn[:, gv]).astype(mdt),
            "b_qk": np.ascontiguousarray(b_qk),
            "b_v_bc": np.tile(b_v, (128, 1)),
            "w_pr": np.ascontiguousarray(w_proj[gq]).astype(mdt),
            "ones": ones,
        })
    return in_maps


def kernel(x, w_attn, b_attn, w_proj, b_proj):
    global LAST_RESULTS
    in_maps = make_in_maps(x, w_attn, b_attn, w_proj)
    nc = build_bass()
    res = run_bass_kernel_spmd(nc, in_maps, list(range(N_CORES)), trace=TRACE)
    LAST_RESULTS = res
    b_proj = np.asarray(b_proj, dtype=np.float32)
    out = np.empty((B, T_FULL, C), dtype=np.float32)
    for b in range(B):
        out[b] = res.results[2 * b]["out"] + res.results[2 * b + 1]["out"] \
            + b_proj[None, :]
    return out


# revision 25
# speedup vs baseline: 3.3330x; 1.1441x over previous
"""Multi-head attention (no mask) on 8 trn2 NeuronCores.

Problem: x[4,2048,1024] @ w_attn[1024,3072] + b_attn -> qkv, 16 heads x 64,
softmax(q k^T / 8) v, merge heads, @ w_proj[1024,1024] + b_proj.

Sharding: core c = (batch b = c//2, head-group g = c%2).  Each core handles
one batch and 8 heads (tensor-parallel over heads), producing a partial
c_proj output; the host adds the two partials per batch plus b_proj.

Device layout (all fp32):
  xT   [C, T]     host-pretransposed activation (c on partitions on chip)
  qT,kT[512, T]   = (x @ w_q/k + b)^T, stored as 4 chunks of [128, T]
                   (each chunk = 2 heads stacked 64+64 on partitions)
  v    [T, 512]   natural layout, 16 chunks of [128, 512]
  S^T  [j, i]     per head via row-tiled matmuls (k^T stationary)
  exp  on ACT with fused 1/8 scale, no max subtraction (scores are O(5))
  den  = sum_j exp via ones-matmuls (col-tiled into 32-row PSUM strips)
  y^T  [d, i]     via v-stationary col-tiled matmuls (2 heads concurrent)
  out  [T, 1024]  = y^T.T @ w_proj chunks, accumulated over 4 dcat chunks
"""

import numpy as np
from contextlib import ExitStack

import concourse.bass as bass
import concourse.tile as tile
from concourse import bacc, mybir
from concourse.bass_utils import run_bass_kernel_spmd

F32 = mybir.dt.float32
EXP = mybir.ActivationFunctionType.Exp

B, T_FULL, C = 4, 2048, 1024
N_HEAD, HEAD_DIM = 16, 64
HPG = 8           # heads per group (per core)
QKD = HPG * HEAD_DIM   # 512: per-core q/k/v width
N_CORES = 8
SCALE = 1.0 / np.sqrt(HEAD_DIM)

# test.py can flip these to get a profile out of the run
TRACE = False
LAST_RESULTS = None


def build_bass(T=T_FULL, use_bf16=True):
    """Build the per-core Bass program (same program for all 8 cores)."""
    NCC = C // 128          # 8 c-chunks
    NTC = T // 128          # t-chunks (16 at full size)
    TH_SIZE = T // 2        # phase-1 t-half
    NI_TH = TH_SIZE // 512 if TH_SIZE >= 512 else 1   # 512-col mm splits
    I_BLK = min(512, T)
    N_I = T // I_BLK        # i-blocks (4 at full size)
    N_PAIR = HPG // 2       # 4 head pairs

    # Bacc (not raw Bass): its compile() runs generate_event_semaphores,
    # which legalizes multi-wait instructions (HW allows 1 wait/inst).
    nc = bacc.Bacc("TRN2", target_bir_lowering=False, debug=False,
                   num_devices=N_CORES)

    mdt = mybir.dt.bfloat16 if use_bf16 else F32
    xT = nc.dram_tensor("xT", [C, T], mdt, kind="ExternalInput").ap()
    w_qk = nc.dram_tensor("w_qk", [C, 2 * QKD], mdt, kind="ExternalInput").ap()
    w_v = nc.dram_tensor("w_v", [C, QKD], mdt, kind="ExternalInput").ap()
    b_qk = nc.dram_tensor("b_qk", [128, 8], F32, kind="ExternalInput").ap()
    b_v_bc = nc.dram_tensor("b_v_bc", [128, QKD], F32, kind="ExternalInput").ap()
    w_pr = nc.dram_tensor("w_pr", [QKD, C], mdt, kind="ExternalInput").ap()
    ones = nc.dram_tensor("ones", [128, 64], F32, kind="ExternalInput").ap()
    out = nc.dram_tensor("out", [T, C], F32, kind="ExternalOutput").ap()
    # DRAM bounce for softmax reciprocals (SBUF sources cannot
    # partition-broadcast, DRAM sources can)
    N_I_ = T // min(512, T)
    rcd = nc.dram_tensor("rc_scratch", [HPG // 2, 2 * N_I_, min(512, T)],
                         F32).ap()

    BF = mybir.dt.bfloat16
    edt = BF if use_bf16 else F32

    with tile.TileContext(nc) as tc, ExitStack() as ctx:
        persist = ctx.enter_context(tc.tile_pool(name="persist", bufs=1))
        qT = persist.tile([128, N_PAIR, T], edt)
        kT = persist.tile([128, N_PAIR, T], edt)
        # v stored 65-wide per head: 64 data cols + a ones column that
        # makes row 64 of each y matmul the softmax denominator
        v = persist.tile([128, NTC, HPG * 65], edt)
        ones_sb = persist.tile([128, 64], edt)
        bqk_sb = persist.tile([128, 8], F32)
        bvbc_sb = persist.tile([128, QKD], F32)

        dvescr = persist.tile([1, 8], F32)
        nc.sync.dma_start(out=bqk_sb[:], in_=b_qk)
        nc.sync.dma_start(out=bvbc_sb[:], in_=b_v_bc)
        # DVE-side fences: TT/TS instructions also hold only one sync
        # wait, so absorb each bias-DMA wait into a tiny copy first
        nc.vector.tensor_copy(dvescr[0:1, 0:1], bqk_sb[0:1, 0:1])
        nc.vector.tensor_copy(dvescr[0:1, 1:2], bvbc_sb[0:1, 0:1])
        if use_bf16:
            nc.gpsimd.dma_start(out=ones_sb[:], in_=ones)  # casts f32->bf16
            ones_f32 = persist.tile([128, 64], F32)
            nc.sync.dma_start(out=ones_f32[:], in_=ones)
        else:
            nc.sync.dma_start(out=ones_sb[:], in_=ones)
            ones_f32 = ones_sb

        yT = persist.tile([128, N_PAIR, T], edt)
        wp_sb = persist.tile([128, N_PAIR, C], edt)

        # ---------------- phase 1a: loads + v projection ----------------
        # Fences: each freshly-DMA'd matmul input gets a 1x1x1 dummy
        # matmul so real matmuls see at most one unobserved semaphore
        # (keeps Bacc's event-semaphore splitting to a minimum).
        xT_r = xT.rearrange("(c p) t -> p c t", p=128)
        with tc.tile_pool(name="ph1w", bufs=1) as ph1w, \
             tc.tile_pool(name="ph1x", bufs=2) as ph1x:
            xt0 = ph1x.tile([128, NCC, TH_SIZE], edt, tag="xt")
            xt1 = ph1x.tile([128, NCC, TH_SIZE], edt, tag="xt")
            wqk_sb = ph1w.tile([128, NCC, 2 * QKD], edt)
            wv_sb = ph1w.tile([128, NCC, QKD], edt)
            nc.sync.dma_start(out=xt0[:], in_=xT_r[:, :, 0:TH_SIZE])
            nc.sync.dma_start(out=wv_sb[:],
                              in_=w_v.rearrange("(c p) n -> p c n", p=128))
            nc.sync.dma_start(out=xt1[:], in_=xT_r[:, :, TH_SIZE:T])
            nc.sync.dma_start(out=wqk_sb[:],
                              in_=w_qk.rearrange("(c p) n -> p c n", p=128))
            nc.sync.dma_start(out=wp_sb[:],
                              in_=w_pr.rearrange("(d p) n -> p d n", p=128))
            xts = [xt0, xt1]
            v_r = v[:, :, :].rearrange("q t (h e) -> q t h e", e=65)
            nc.vector.memset(v_r[:, :, :, 64:65], 1.0)

            with tc.tile_pool(name="pp_v", bufs=2, space="PSUM") as pp_v:
                fence_ps = pp_v.tile([1, 8], F32, tag="fence")
                nc.tensor.matmul(fence_ps[0:1, 0:1], ones_sb[0:1, 0:1],
                                 ones_sb[0:1, 0:1], start=True, stop=True)
                if use_bf16:
                    nc.tensor.matmul(fence_ps[0:1, 1:2], ones_f32[0:1, 0:1],
                                     ones_f32[0:1, 0:1], start=True, stop=True)
                for fi, ft in enumerate((xt0, wv_sb, xt1, wqk_sb, wp_sb)):
                    nc.tensor.matmul(fence_ps[0:1, 2 + fi:3 + fi],
                                     ft[0:1, 0, 0:1], ones_sb[0:1, 0:1],
                                     start=True, stop=True)
                for th in range(2):
                    for tcl in range(TH_SIZE // 128):
                        tg = th * (TH_SIZE // 128) + tcl
                        psv = pp_v.tile([128, QKD], F32, tag="psv")
                        for c in range(NCC):
                            nc.tensor.matmul(
                                psv[:],
                                xts[th][:, c, tcl * 128:(tcl + 1) * 128],
                                wv_sb[:, c, :],
                                start=(c == 0), stop=(c == NCC - 1))
                        nc.vector.tensor_add(
                            v[:, tg, :].rearrange(
                                "q (h e) -> q h e", e=65)[:, :, 0:64],
                            psv[:].rearrange("q (h e) -> q h e", e=64),
                            bvbc_sb[:].rearrange("q (h e) -> q h e", e=64))

            # ------- phase 1b/2: per-pair q/k projection + attention -------
            # Emitting each pair's q/k projection right before its
            # attention lets the scheduler fill PE idle slots (attention
            # is exp/ACT-paced) with the next pair's projection matmuls.
            def qk_proj(p, qk_pool):
                for dcq in (p, p + 4):
                    for th in range(2):
                        for i2 in range(NI_TH):
                            lo = th * TH_SIZE + i2 * 512
                            w = min(512, TH_SIZE)
                            isl = slice(i2 * 512, i2 * 512 + w)
                            ps = qk_pool.tile([128, 512], F32, tag="psqk")
                            for c in range(NCC):
                                nc.tensor.matmul(
                                    ps[:, 0:w],
                                    wqk_sb[:, c, dcq * 128:(dcq + 1) * 128],
                                    xts[th][:, c, isl],
                                    start=(c == 0), stop=(c == NCC - 1))
                            dst = (qT if dcq < 4 else kT)[:, p, lo:lo + w]
                            nc.vector.tensor_scalar_add(
                                dst, ps[:, 0:w], bqk_sb[:, dcq:dcq + 1])

        # ---------------- phase 2: attention ----------------
        # Per head: S^T via K=64 matmuls (row-pair per es grain), then
        # y accumulation with M=65 single-tile matmuls whose 65th lhsT
        # column is all-ones -> row 64 of the y accumulator is the
        # softmax denominator (free: matmul time is N-bound).
            with tc.tile_pool(name="pp_qk", bufs=2, space="PSUM") as qk_pool, \
                 tc.tile_pool(name="att_s", bufs=2, space="PSUM") as s_pool, \
                 tc.tile_pool(name="att_y", bufs=2, space="PSUM") as y_pool, \
                 tc.tile_pool(name="att_es", bufs=3) as es_pool, \
                 tc.tile_pool(name="att_yr", bufs=4) as yr_pool, \
                 tc.tile_pool(name="att_st", bufs=4) as st_pool, \
                 tc.tile_pool(name="att_cl", bufs=2) as cl_pool, \
                 tc.tile_pool(name="att_rc", bufs=2) as rc_pool, \
                 tc.tile_pool(name="att_bc", bufs=4) as bc_pool:
                for p in range(N_PAIR):
                    qk_proj(p, qk_pool)
                    yraw_a = yr_pool.tile([64, T], edt, tag="yraw")
                    yraw_b = yr_pool.tile([64, T], edt, tag="yraw")
                    yraws = [yraw_a, yraw_b]
                    coll = cl_pool.tile([2 * N_I, I_BLK], F32, tag="coll")
                    for i in range(N_I):
                        isl = slice(i * I_BLK, (i + 1) * I_BLK)
                        y_a = y_pool.tile([65, I_BLK], F32, tag="y")
                        y_b = y_pool.tile([65, I_BLK], F32, tag="y")
                        ys = [y_a, y_b]
                        for j in range(NTC):
                            jsl = slice(j * 128, (j + 1) * 128)
                            s = s_pool.tile([128, 2 * I_BLK], F32, tag="s")
                            nc.tensor.matmul(s[:, 0:I_BLK],
                                             kT[0:64, p, jsl], qT[0:64, p, isl],
                                             start=True, stop=True)
                            nc.tensor.matmul(s[:, I_BLK:2 * I_BLK],
                                             kT[64:128, p, jsl],
                                             qT[64:128, p, isl],
                                             start=True, stop=True)
                            es = es_pool.tile([128, 2 * I_BLK], edt, tag="es")
                            nc.scalar.activation(es[:], s[:], EXP, scale=SCALE)
                            for hl in range(2):
                                h = 2 * p + hl
                                nc.tensor.matmul(
                                    ys[hl][0:65, :],
                                    v[:, j, 65 * h:65 * h + 65],
                                    es[:, hl * I_BLK:(hl + 1) * I_BLK],
                                    start=(j == 0), stop=(j == NTC - 1))
                        for hl in range(2):
                            st = st_pool.tile([65, I_BLK], F32, tag="st")
                            nc.vector.tensor_copy(st[64:65, :],
                                                  ys[hl][64:65, :])
                            nc.vector.tensor_copy(yraws[hl][:, isl],
                                                  ys[hl][0:64, :])
                            # move denominator row to its own partition
                            r = hl * N_I + i
                            nc.gpsimd.dma_start(out=coll[r:r + 1, :],
                                                in_=st[64:65, :])
                    rc = rc_pool.tile([2 * N_I, I_BLK], F32, tag="rc")
                    nc.vector.reciprocal(rc[:], coll[:])
                    nc.gpsimd.dma_start(out=rcd[p], in_=rc[:])
                    for hl in range(2):
                        for i in range(N_I):
                            isl = slice(i * I_BLK, (i + 1) * I_BLK)
                            r = hl * N_I + i
                            bcast = bc_pool.tile([64, I_BLK], edt, tag="bcast")
                            rrow = rcd[p, r, :]
                            rbc = bass.AP(tensor=rrow.tensor,
                                          offset=rrow.offset,
                                          ap=[[0, 64]] + list(rrow.ap))
                            nc.gpsimd.dma_start(out=bcast[:], in_=rbc)
                            if hl == 0:
                                nc.vector.tensor_mul(yT[0:64, p, isl],
                                                     yraws[0][:, isl],
                                                     bcast[:])
                            else:
                                ybst = bc_pool.tile([64, I_BLK], edt,
                                                    tag="ybst")
                                nc.vector.tensor_mul(ybst[:],
                                                     yraws[1][:, isl],
                                                     bcast[:])
                                nc.gpsimd.dma_start(out=yT[64:128, p, isl],
                                                    in_=ybst[:])

        # ---------------- phase 3: output projection ----------------
        with tc.tile_pool(name="ph3o", bufs=3) as ph3o, \
             tc.tile_pool(name="pp_o", bufs=3, space="PSUM") as pp_o:
            for tcl in range(NTC):
                ps = pp_o.tile([128, C], F32, tag="pso")
                for d in range(N_PAIR):
                    for n2 in range(C // 512):
                        nsl = slice(n2 * 512, (n2 + 1) * 512)
                        nc.tensor.matmul(
                            ps[:, nsl],
                            yT[:, d, tcl * 128:(tcl + 1) * 128],
                            wp_sb[:, d, nsl],
                            start=(d == 0), stop=(d == N_PAIR - 1))
                os = ph3o.tile([128, C], F32, tag="os")
                # absorb the WAR wait on the slot's previous out-DMA
                nc.vector.memset(os[0:1, 0:1], 0.0)
                nc.vector.tensor_copy(os[:], ps[:])
                nc.sync.dma_start(out=out[tcl * 128:(tcl + 1) * 128, :],
                                  in_=os[:])

    nc.compile()
    return nc


def make_in_maps(x, w_attn, b_attn, w_proj, T=T_FULL, use_bf16=True):
    """Host-side sharding: per-core input dict."""
    import ml_dtypes
    mdt = ml_dtypes.bfloat16 if use_bf16 else np.float32
    x = np.asarray(x, dtype=np.float32)
    w_attn = np.asarray(w_attn, dtype=np.float32)
    b_attn = np.asarray(b_attn, dtype=np.float32)
    w_proj = np.asarray(w_proj, dtype=np.float32)
    in_maps = []
    ones = np.ones((128, 64), dtype=np.float32)
    for core in range(N_CORES):
        b, g = core // 2, core % 2
        gq = slice(g * QKD, (g + 1) * QKD)
        gk = slice(C + g * QKD, C + (g + 1) * QKD)
        gv = slice(2 * C + g * QKD, 2 * C + (g + 1) * QKD)
        w_qk = np.concatenate([w_attn[:, gq], w_attn[:, gk]], axis=1)
        b_q = b_attn[gq]
        b_k = b_attn[gk]
        b_v = b_attn[gv]
        b_qk = np.stack([b_q.reshape(4, 128), b_k.reshape(4, 128)],
                        axis=0).reshape(8, 128).T.copy()   # [128, 8]
        in_maps.append({
            "xT": np.ascontiguousarray(x[b, :T].T).astype(mdt),
            "w_qk": np.ascontiguousarray(w_qk).astype(mdt),
            "w_v": np.ascontiguousarray(w_attn[:, gv]).astype(mdt),
            "b_qk": np.ascontiguousarray(b_qk),
            "b_v_bc": np.tile(b_v, (128, 1)),
            "w_pr": np.ascontiguousarray(w_proj[gq]).astype(mdt),
            "ones": ones,
        })
    return in_maps


def kernel(x, w_attn, b_attn, w_proj, b_proj):
    global LAST_RESULTS
    in_maps = make_in_maps(x, w_attn, b_attn, w_proj)
    nc = build_bass()
    res = run_bass_kernel_spmd(nc, in_maps, list(range(N_CORES)), trace=TRACE)
    LAST_RESULTS = res
    b_proj = np.asarray(b_proj, dtype=np.float32)
    out = np.empty((B, T_FULL, C), dtype=np.float32)
    for b in range(B):
        out[b] = res.results[2 * b]["out"] + res.results[2 * b + 1]["out"] \
            + b_proj[None, :]
    return out


# revision 27
# speedup vs baseline: 3.4227x; 1.0269x over previous
"""Multi-head attention (no mask) on 8 trn2 NeuronCores.

Problem: x[4,2048,1024] @ w_attn[1024,3072] + b_attn -> qkv, 16 heads x 64,
softmax(q k^T / 8) v, merge heads, @ w_proj[1024,1024] + b_proj.

Sharding: core c = (batch b = c//2, head-group g = c%2).  Each core handles
one batch and 8 heads (tensor-parallel over heads), producing a partial
c_proj output; the host adds the two partials per batch plus b_proj.

Device layout (all fp32):
  xT   [C, T]     host-pretransposed activation (c on partitions on chip)
  qT,kT[512, T]   = (x @ w_q/k + b)^T, stored as 4 chunks of [128, T]
                   (each chunk = 2 heads stacked 64+64 on partitions)
  v    [T, 512]   natural layout, 16 chunks of [128, 512]
  S^T  [j, i]     per head via row-tiled matmuls (k^T stationary)
  exp  on ACT with fused 1/8 scale, no max subtraction (scores are O(5))
  den  = sum_j exp via ones-matmuls (col-tiled into 32-row PSUM strips)
  y^T  [d, i]     via v-stationary col-tiled matmuls (2 heads concurrent)
  out  [T, 1024]  = y^T.T @ w_proj chunks, accumulated over 4 dcat chunks
"""

import numpy as np
from contextlib import ExitStack

import concourse.bass as bass
import concourse.tile as tile
from concourse import bacc, mybir
from concourse.bass_utils import run_bass_kernel_spmd

F32 = mybir.dt.float32
EXP = mybir.ActivationFunctionType.Exp

B, T_FULL, C = 4, 2048, 1024
N_HEAD, HEAD_DIM = 16, 64
HPG = 8           # heads per group (per core)
QKD = HPG * HEAD_DIM   # 512: per-core q/k/v width
N_CORES = 8
SCALE = 1.0 / np.sqrt(HEAD_DIM)

# test.py can flip these to get a profile out of the run
TRACE = False
LAST_RESULTS = None


def build_bass(T=T_FULL, use_bf16=True):
    """Build the per-core Bass program (same program for all 8 cores)."""
    NCC = C // 128          # 8 c-chunks
    NTC = T // 128          # t-chunks (16 at full size)
    TH_SIZE = T // 2        # phase-1 t-half
    NI_TH = TH_SIZE // 512 if TH_SIZE >= 512 else 1   # 512-col mm splits
    I_BLK = min(512, T)
    N_I = T // I_BLK        # i-blocks (4 at full size)
    N_PAIR = HPG // 2       # 4 head pairs

    # Bacc (not raw Bass): its compile() runs generate_event_semaphores,
    # which legalizes multi-wait instructions (HW allows 1 wait/inst).
    nc = bacc.Bacc("TRN2", target_bir_lowering=False, debug=False,
                   num_devices=N_CORES)

    mdt = mybir.dt.bfloat16 if use_bf16 else F32
    xT = nc.dram_tensor("xT", [C, T], mdt, kind="ExternalInput").ap()
    w_qk = nc.dram_tensor("w_qk", [C, 2 * QKD], mdt, kind="ExternalInput").ap()
    w_v = nc.dram_tensor("w_v", [C, QKD], mdt, kind="ExternalInput").ap()
    b_qk = nc.dram_tensor("b_qk", [128, 8], F32, kind="ExternalInput").ap()
    b_v_bc = nc.dram_tensor("b_v_bc", [128, QKD], F32, kind="ExternalInput").ap()
    w_pr = nc.dram_tensor("w_pr", [QKD, C], mdt, kind="ExternalInput").ap()
    ones = nc.dram_tensor("ones", [128, 64], F32, kind="ExternalInput").ap()
    out = nc.dram_tensor("out", [T, C], F32, kind="ExternalOutput").ap()
    # DRAM bounce for softmax reciprocals (SBUF sources cannot
    # partition-broadcast, DRAM sources can)
    N_I_ = T // min(512, T)
    rcd = nc.dram_tensor("rc_scratch", [HPG // 2, 2 * N_I_, min(512, T)],
                         F32).ap()

    BF = mybir.dt.bfloat16
    edt = BF if use_bf16 else F32

    with tile.TileContext(nc) as tc, ExitStack() as ctx:
        persist = ctx.enter_context(tc.tile_pool(name="persist", bufs=1))
        qT = persist.tile([128, N_PAIR, T], edt)
        kT = persist.tile([128, N_PAIR, T], edt)
        # v stored 65-wide per head: 64 data cols + a ones column that
        # makes row 64 of each y matmul the softmax denominator
        v = persist.tile([128, NTC, HPG * 65], edt)
        ones_sb = persist.tile([128, 64], edt)
        bqk_sb = persist.tile([128, 8], F32)
        bvbc_sb = persist.tile([128, QKD], F32)

        dvescr = persist.tile([1, 8], F32)
        nc.sync.dma_start(out=bqk_sb[:], in_=b_qk)
        nc.sync.dma_start(out=bvbc_sb[:], in_=b_v_bc)
        # DVE-side fences: TT/TS instructions also hold only one sync
        # wait, so absorb each bias-DMA wait into a tiny copy first
        nc.vector.tensor_copy(dvescr[0:1, 0:1], bqk_sb[0:1, 0:1])
        nc.vector.tensor_copy(dvescr[0:1, 1:2], bvbc_sb[0:1, 0:1])
        if use_bf16:
            nc.gpsimd.dma_start(out=ones_sb[:], in_=ones)  # casts f32->bf16
            ones_f32 = persist.tile([128, 64], F32)
            nc.sync.dma_start(out=ones_f32[:], in_=ones)
        else:
            nc.sync.dma_start(out=ones_sb[:], in_=ones)
            ones_f32 = ones_sb

        yT = persist.tile([128, N_PAIR, T], edt)
        wp_sb = persist.tile([128, N_PAIR, C], edt)

        # ---------------- phase 1a: loads + v projection ----------------
        # Fences: each freshly-DMA'd matmul input gets a 1x1x1 dummy
        # matmul so real matmuls see at most one unobserved semaphore
        # (keeps Bacc's event-semaphore splitting to a minimum).
        xT_r = xT.rearrange("(c p) t -> p c t", p=128)
        with tc.tile_pool(name="ph1w", bufs=1) as ph1w, \
             tc.tile_pool(name="ph1x", bufs=2) as ph1x:
            xt0 = ph1x.tile([128, NCC, TH_SIZE], edt, tag="xt")
            xt1 = ph1x.tile([128, NCC, TH_SIZE], edt, tag="xt")
            wqk_sb = ph1w.tile([128, NCC, 2 * QKD], edt)
            wv_sb = ph1w.tile([128, NCC, QKD], edt)
            nc.sync.dma_start(out=xt0[:], in_=xT_r[:, :, 0:TH_SIZE])
            nc.sync.dma_start(out=wv_sb[:],
                              in_=w_v.rearrange("(c p) n -> p c n", p=128))
            nc.sync.dma_start(out=xt1[:], in_=xT_r[:, :, TH_SIZE:T])
            nc.sync.dma_start(out=wqk_sb[:],
                              in_=w_qk.rearrange("(c p) n -> p c n", p=128))
            nc.sync.dma_start(out=wp_sb[:],
                              in_=w_pr.rearrange("(d p) n -> p d n", p=128))
            xts = [xt0, xt1]
            v_r = v[:, :, :].rearrange("q t (h e) -> q t h e", e=65)
            nc.vector.memset(v_r[:, :, :, 64:65], 1.0)

            # v projection is emitted per-tc inside pair 0's first
            # j-loop (tc == j there), so exp can start ~30us earlier.
            def v_proj_tc(tg, ps_pool):
                psv = ps_pool.tile([128, QKD], F32, tag="ps")
                th = tg // (TH_SIZE // 128)
                tcl = tg % (TH_SIZE // 128)
                for c in range(NCC):
                    nc.tensor.matmul(
                        psv[:],
                        xts[th][:, c, tcl * 128:(tcl + 1) * 128],
                        wv_sb[:, c, :],
                        start=(c == 0), stop=(c == NCC - 1))
                nc.vector.tensor_add(
                    v[:, tg, :].rearrange(
                        "q (h e) -> q h e", e=65)[:, :, 0:64],
                    psv[:].rearrange("q (h e) -> q h e", e=64),
                    bvbc_sb[:].rearrange("q (h e) -> q h e", e=64))

            # ------- phase 1b/2: per-pair q/k projection + attention -------
            # Emitting each pair's q/k projection right before its
            # attention lets the scheduler fill PE idle slots (attention
            # is exp/ACT-paced) with the next pair's projection matmuls.
            def qk_proj(p, qk_pool):
                for dcq in (p, p + 4):
                    for th in range(2):
                        for i2 in range(NI_TH):
                            lo = th * TH_SIZE + i2 * 512
                            w = min(512, TH_SIZE)
                            isl = slice(i2 * 512, i2 * 512 + w)
                            ps = qk_pool.tile([128, 512], F32, tag="ps")
                            for c in range(NCC):
                                nc.tensor.matmul(
                                    ps[:, 0:w],
                                    wqk_sb[:, c, dcq * 128:(dcq + 1) * 128],
                                    xts[th][:, c, isl],
                                    start=(c == 0), stop=(c == NCC - 1))
                            dst = (qT if dcq < 4 else kT)[:, p, lo:lo + w]
                            nc.vector.tensor_scalar_add(
                                dst, ps[:, 0:w], bqk_sb[:, dcq:dcq + 1])

        # ---------------- phase 2: attention ----------------
        # Per head: S^T via K=64 matmuls (row-pair per es grain), then
        # y accumulation with M=65 single-tile matmuls whose 65th lhsT
        # column is all-ones -> row 64 of the y accumulator is the
        # softmax denominator (free: matmul time is N-bound).
            with tc.tile_pool(name="pp_qk", bufs=2, space="PSUM") as qk_pool, \
                 tc.tile_pool(name="att_s", bufs=2, space="PSUM") as s_pool, \
                 tc.tile_pool(name="att_y", bufs=2, space="PSUM") as y_pool, \
                 tc.tile_pool(name="att_es", bufs=3) as es_pool, \
                 tc.tile_pool(name="att_yr", bufs=4) as yr_pool, \
                 tc.tile_pool(name="att_st", bufs=4) as st_pool, \
                 tc.tile_pool(name="att_cl", bufs=3) as cl_pool, \
                 tc.tile_pool(name="att_rc", bufs=3) as rc_pool, \
                 tc.tile_pool(name="att_bc", bufs=4) as bc_pool:
                fence_ps = y_pool.tile([1, 8], F32, tag="y")
                nc.tensor.matmul(fence_ps[0:1, 0:1], ones_sb[0:1, 0:1],
                                 ones_sb[0:1, 0:1], start=True, stop=True)
                if use_bf16:
                    nc.tensor.matmul(fence_ps[0:1, 1:2], ones_f32[0:1, 0:1],
                                     ones_f32[0:1, 0:1], start=True, stop=True)
                for fi, ft in enumerate((xt0, wv_sb, xt1, wqk_sb, wp_sb)):
                    nc.tensor.matmul(fence_ps[0:1, 2 + fi:3 + fi],
                                     ft[0:1, 0, 0:1], ones_sb[0:1, 0:1],
                                     start=True, stop=True)
                for p in range(N_PAIR):
                    qk_proj(p, qk_pool)
                    yraw_a = yr_pool.tile([64, T], edt, tag="yraw")
                    yraw_b = yr_pool.tile([64, T], edt, tag="yraw")
                    yraws = [yraw_a, yraw_b]
                    for i in range(N_I):
                        isl = slice(i * I_BLK, (i + 1) * I_BLK)
                        y_a = y_pool.tile([65, I_BLK], F32, tag="y")
                        y_b = y_pool.tile([65, I_BLK], F32, tag="y")
                        ys = [y_a, y_b]
                        for j in range(NTC):
                            jsl = slice(j * 128, (j + 1) * 128)
                            s = s_pool.tile([128, 2 * I_BLK], F32, tag="s")
                            nc.tensor.matmul(s[:, 0:I_BLK],
                                             kT[0:64, p, jsl], qT[0:64, p, isl],
                                             start=True, stop=True)
                            nc.tensor.matmul(s[:, I_BLK:2 * I_BLK],
                                             kT[64:128, p, jsl],
                                             qT[64:128, p, isl],
                                             start=True, stop=True)
                            if p == 0 and i == 0:
                                v_proj_tc(j, qk_pool)
                            es = es_pool.tile([128, 2 * I_BLK], edt, tag="es")
                            nc.scalar.activation(es[:], s[:], EXP, scale=SCALE)
                            for hl in range(2):
                                h = 2 * p + hl
                                nc.tensor.matmul(
                                    ys[hl][0:65, :],
                                    v[:, j, 65 * h:65 * h + 65],
                                    es[:, hl * I_BLK:(hl + 1) * I_BLK],
                                    start=(j == 0), stop=(j == NTC - 1))
                        # per-i denominator handling so yT streams out
                        # (keeps the output projection from piling up at
                        # the very end of attention)
                        coll = cl_pool.tile([2, I_BLK], F32, tag="coll")
                        for hl in range(2):
                            st = st_pool.tile([65, I_BLK], F32, tag="st")
                            nc.vector.tensor_copy(st[64:65, :],
                                                  ys[hl][64:65, :])
                            nc.vector.tensor_copy(yraws[hl][:, isl],
                                                  ys[hl][0:64, :])
                            # move denominator row to its own partition
                            nc.gpsimd.dma_start(out=coll[hl:hl + 1, :],
                                                in_=st[64:65, :])
                        rc = rc_pool.tile([2, I_BLK], F32, tag="rc")
                        nc.vector.reciprocal_approx_fast(rc[:], coll[:])
                        nc.gpsimd.dma_start(out=rcd[p, 2 * i:2 * i + 2],
                                            in_=rc[:])
                        for hl in range(2):
                            bcast = bc_pool.tile([64, I_BLK], edt, tag="bcast")
                            rrow = rcd[p, 2 * i + hl, :]
                            rbc = bass.AP(tensor=rrow.tensor,
                                          offset=rrow.offset,
                                          ap=[[0, 64]] + list(rrow.ap))
                            nc.gpsimd.dma_start(out=bcast[:], in_=rbc)
                            if hl == 0:
                                nc.vector.tensor_mul(yT[0:64, p, isl],
                                                     yraws[0][:, isl],
                                                     bcast[:])
                            else:
                                ybst = bc_pool.tile([64, I_BLK], edt,
                                                    tag="ybst")
                                nc.vector.tensor_mul(ybst[:],
                                                     yraws[1][:, isl],
                                                     bcast[:])
                                nc.gpsimd.dma_start(out=yT[64:128, p, isl],
                                                    in_=ybst[:])

        # ---------------- phase 3: output projection ----------------
        with tc.tile_pool(name="ph3o", bufs=3) as ph3o, \
             tc.tile_pool(name="pp_o", bufs=3, space="PSUM") as pp_o:
            for tcl in range(NTC):
                ps = pp_o.tile([128, C], F32, tag="pso")
                for d in range(N_PAIR):
                    for n2 in range(C // 512):
                        nsl = slice(n2 * 512, (n2 + 1) * 512)
                        nc.tensor.matmul(
                            ps[:, nsl],
                            yT[:, d, tcl * 128:(tcl + 1) * 128],
                            wp_sb[:, d, nsl],
                            start=(d == 0), stop=(d == N_PAIR - 1))
                os = ph3o.tile([128, C], F32, tag="os")
                # absorb the WAR wait on the slot's previous out-DMA
                nc.vector.memset(os[0:1, 0:1], 0.0)
                nc.vector.tensor_copy(os[:], ps[:])
                nc.sync.dma_start(out=out[tcl * 128:(tcl + 1) * 128, :],
                                  in_=os[:])

    nc.compile()
    return nc


def make_in_maps(x, w_attn, b_attn, w_proj, T=T_FULL, use_bf16=True):
    """Host-side sharding: per-core input dict."""
    import ml_dtypes
    mdt = ml_dtypes.bfloat16 if use_bf16 else np.float32
    x = np.asarray(x, dtype=np.float32)
    w_attn = np.asarray(w_attn, dtype=np.float32)
    b_attn = np.asarray(b_attn, dtype=np.float32)
    w_proj = np.asarray(w_proj, dtype=np.float32)
    in_maps = []
    ones = np.ones((128, 64), dtype=np.float32)
    for core in range(N_CORES):
        b, g = core // 2, core % 2
        gq = slice(g * QKD, (g + 1) * QKD)
        gk = slice(C + g * QKD, C + (g + 1) * QKD)
        gv = slice(2 * C + g * QKD, 2 * C + (g + 1) * QKD)
        w_qk = np.concatenate([w_attn[:, gq], w_attn[:, gk]], axis=1)
        b_q = b_attn[gq]
        b_k = b_attn[gk]
        b_v = b_attn[gv]
        b_qk = np.stack([b_q.reshape(4, 128), b_k.reshape(4, 128)],
                        axis=0).reshape(8, 128).T.copy()   # [128, 8]
        in_maps.append({
            "xT": np.ascontiguousarray(x[b, :T].T).astype(mdt),
            "w_qk": np.ascontiguousarray(w_qk).astype(mdt),
            "w_v": np.ascontiguousarray(w_attn[:, gv]).astype(mdt),
            "b_qk": np.ascontiguousarray(b_qk),
            "b_v_bc": np.tile(b_v, (128, 1)),
            "w_pr": np.ascontiguousarray(w_proj[gq]).astype(mdt),
            "ones": ones,
        })
    return in_maps


def kernel(x, w_attn, b_attn, w_proj, b_proj):
    global LAST_RESULTS
    in_maps = make_in_maps(x, w_attn, b_attn, w_proj)
    nc = build_bass()
    res = run_bass_kernel_spmd(nc, in_maps, list(range(N_CORES)), trace=TRACE)
    LAST_RESULTS = res
    b_proj = np.asarray(b_proj, dtype=np.float32)
    out = np.empty((B, T_FULL, C), dtype=np.float32)
    for b in range(B):
        out[b] = res.results[2 * b]["out"] + res.results[2 * b + 1]["out"] \
            + b_proj[None, :]
    return out


# revision 28
# speedup vs baseline: 3.4538x; 1.0091x over previous
"""Multi-head attention (no mask) on 8 trn2 NeuronCores.

Problem: x[4,2048,1024] @ w_attn[1024,3072] + b_attn -> qkv, 16 heads x 64,
softmax(q k^T / 8) v, merge heads, @ w_proj[1024,1024] + b_proj.

Sharding: core c = (batch b = c//2, head-group g = c%2).  Each core handles
one batch and 8 heads (tensor-parallel over heads), producing a partial
c_proj output; the host adds the two partials per batch plus b_proj.

Device layout (all fp32):
  xT   [C, T]     host-pretransposed activation (c on partitions on chip)
  qT,kT[512, T]   = (x @ w_q/k + b)^T, stored as 4 chunks of [128, T]
                   (each chunk = 2 heads stacked 64+64 on partitions)
  v    [T, 512]   natural layout, 16 chunks of [128, 512]
  S^T  [j, i]     per head via row-tiled matmuls (k^T stationary)
  exp  on ACT with fused 1/8 scale, no max subtraction (scores are O(5))
  den  = sum_j exp via ones-matmuls (col-tiled into 32-row PSUM strips)
  y^T  [d, i]     via v-stationary col-tiled matmuls (2 heads concurrent)
  out  [T, 1024]  = y^T.T @ w_proj chunks, accumulated over 4 dcat chunks
"""

import numpy as np
from contextlib import ExitStack

import concourse.bass as bass
import concourse.tile as tile
from concourse import bacc, mybir
from concourse.bass_utils import run_bass_kernel_spmd

F32 = mybir.dt.float32
EXP = mybir.ActivationFunctionType.Exp

B, T_FULL, C = 4, 2048, 1024
N_HEAD, HEAD_DIM = 16, 64
HPG = 8           # heads per group (per core)
QKD = HPG * HEAD_DIM   # 512: per-core q/k/v width
N_CORES = 8
SCALE = 1.0 / np.sqrt(HEAD_DIM)

# test.py can flip these to get a profile out of the run
TRACE = False
LAST_RESULTS = None


def build_bass(T=T_FULL, use_bf16=True):
    """Build the per-core Bass program (same program for all 8 cores)."""
    NCC = C // 128          # 8 c-chunks
    NTC = T // 128          # t-chunks (16 at full size)
    TH_SIZE = T // 2        # phase-1 t-half
    NI_TH = TH_SIZE // 512 if TH_SIZE >= 512 else 1   # 512-col mm splits
    I_BLK = min(512, T)
    N_I = T // I_BLK        # i-blocks (4 at full size)
    N_PAIR = HPG // 2       # 4 head pairs

    # Bacc (not raw Bass): its compile() runs generate_event_semaphores,
    # which legalizes multi-wait instructions (HW allows 1 wait/inst).
    nc = bacc.Bacc("TRN2", target_bir_lowering=False, debug=False,
                   num_devices=N_CORES)

    mdt = mybir.dt.bfloat16 if use_bf16 else F32
    xT = nc.dram_tensor("xT", [C, T], mdt, kind="ExternalInput").ap()
    w_qk = nc.dram_tensor("w_qk", [C, 2 * QKD], mdt, kind="ExternalInput").ap()
    w_v = nc.dram_tensor("w_v", [C, QKD], mdt, kind="ExternalInput").ap()
    b_qk = nc.dram_tensor("b_qk", [128, 8], F32, kind="ExternalInput").ap()
    b_v_bc = nc.dram_tensor("b_v_bc", [128, QKD], F32, kind="ExternalInput").ap()
    w_pr = nc.dram_tensor("w_pr", [QKD, C], mdt, kind="ExternalInput").ap()
    ones = nc.dram_tensor("ones", [128, 64], F32, kind="ExternalInput").ap()
    out = nc.dram_tensor("out", [T, C], F32, kind="ExternalOutput").ap()
    # DRAM bounce for softmax reciprocals (SBUF sources cannot
    # partition-broadcast, DRAM sources can)
    N_I_ = T // min(512, T)
    rcd = nc.dram_tensor("rc_scratch", [HPG // 2, 2 * N_I_, min(512, T)],
                         F32).ap()

    BF = mybir.dt.bfloat16
    edt = BF if use_bf16 else F32

    with tile.TileContext(nc) as tc, ExitStack() as ctx:
        persist = ctx.enter_context(tc.tile_pool(name="persist", bufs=1))
        qT = persist.tile([128, N_PAIR, T], edt)
        kT = persist.tile([128, N_PAIR, T], edt)
        # v stored 65-wide per head: 64 data cols + a ones column that
        # makes row 64 of each y matmul the softmax denominator
        v = persist.tile([128, NTC, HPG * 65], edt)
        ones_sb = persist.tile([128, 64], edt)
        bqk_sb = persist.tile([128, 8], F32)
        bvbc_sb = persist.tile([128, QKD], F32)

        dvescr = persist.tile([1, 8], F32)
        nc.sync.dma_start(out=bqk_sb[:], in_=b_qk)
        nc.sync.dma_start(out=bvbc_sb[:], in_=b_v_bc)
        # DVE-side fences: TT/TS instructions also hold only one sync
        # wait, so absorb each bias-DMA wait into a tiny copy first
        nc.vector.tensor_copy(dvescr[0:1, 0:1], bqk_sb[0:1, 0:1])
        nc.vector.tensor_copy(dvescr[0:1, 1:2], bvbc_sb[0:1, 0:1])
        if use_bf16:
            nc.gpsimd.dma_start(out=ones_sb[:], in_=ones)  # casts f32->bf16
            ones_f32 = persist.tile([128, 64], F32)
            nc.sync.dma_start(out=ones_f32[:], in_=ones)
        else:
            nc.sync.dma_start(out=ones_sb[:], in_=ones)
            ones_f32 = ones_sb

        yT = persist.tile([128, N_PAIR, T], edt)
        wp_sb = persist.tile([128, N_PAIR, C], edt)

        # ---------------- phase 1a: loads + v projection ----------------
        # Fences: each freshly-DMA'd matmul input gets a 1x1x1 dummy
        # matmul so real matmuls see at most one unobserved semaphore
        # (keeps Bacc's event-semaphore splitting to a minimum).
        xT_r = xT.rearrange("(c p) t -> p c t", p=128)
        with tc.tile_pool(name="ph1w", bufs=1) as ph1w, \
             tc.tile_pool(name="ph1x", bufs=2) as ph1x:
            xt0 = ph1x.tile([128, NCC, TH_SIZE], edt, tag="xt")
            xt1 = ph1x.tile([128, NCC, TH_SIZE], edt, tag="xt")
            wqk_sb = ph1w.tile([128, NCC, 2 * QKD], edt)
            wv_sb = ph1w.tile([128, NCC, QKD], edt)
            nc.sync.dma_start(out=wqk_sb[:],
                              in_=w_qk.rearrange("(c p) n -> p c n", p=128))
            nc.sync.dma_start(out=xt0[:], in_=xT_r[:, :, 0:TH_SIZE])
            nc.sync.dma_start(out=wv_sb[:],
                              in_=w_v.rearrange("(c p) n -> p c n", p=128))
            nc.sync.dma_start(out=xt1[:], in_=xT_r[:, :, TH_SIZE:T])
            nc.sync.dma_start(out=wp_sb[:],
                              in_=w_pr.rearrange("(d p) n -> p d n", p=128))
            xts = [xt0, xt1]
            v_r = v[:, :, :].rearrange("q t (h e) -> q t h e", e=65)
            nc.vector.memset(v_r[:, :, :, 64:65], 1.0)

            # v projection is emitted per-tc inside pair 0's first
            # j-loop (tc == j there), so exp can start ~30us earlier.
            def v_proj_tc(tg, ps_pool):
                psv = ps_pool.tile([128, QKD], F32, tag="ps")
                th = tg // (TH_SIZE // 128)
                tcl = tg % (TH_SIZE // 128)
                for c in range(NCC):
                    nc.tensor.matmul(
                        psv[:],
                        xts[th][:, c, tcl * 128:(tcl + 1) * 128],
                        wv_sb[:, c, :],
                        start=(c == 0), stop=(c == NCC - 1))
                nc.vector.tensor_add(
                    v[:, tg, :].rearrange(
                        "q (h e) -> q h e", e=65)[:, :, 0:64],
                    psv[:].rearrange("q (h e) -> q h e", e=64),
                    bvbc_sb[:].rearrange("q (h e) -> q h e", e=64))

            # ------- phase 1b/2: per-pair q/k projection + attention -------
            # Emitting each pair's q/k projection right before its
            # attention lets the scheduler fill PE idle slots (attention
            # is exp/ACT-paced) with the next pair's projection matmuls.
            def qk_proj(p, qk_pool):
                for dcq in (p, p + 4):
                    for th in range(2):
                        for i2 in range(NI_TH):
                            lo = th * TH_SIZE + i2 * 512
                            w = min(512, TH_SIZE)
                            isl = slice(i2 * 512, i2 * 512 + w)
                            ps = qk_pool.tile([128, 512], F32, tag="ps")
                            for c in range(NCC):
                                nc.tensor.matmul(
                                    ps[:, 0:w],
                                    wqk_sb[:, c, dcq * 128:(dcq + 1) * 128],
                                    xts[th][:, c, isl],
                                    start=(c == 0), stop=(c == NCC - 1))
                            dst = (qT if dcq < 4 else kT)[:, p, lo:lo + w]
                            nc.vector.tensor_scalar_add(
                                dst, ps[:, 0:w], bqk_sb[:, dcq:dcq + 1])

        # ---------------- phase 2: attention ----------------
        # Per head: S^T via K=64 matmuls (row-pair per es grain), then
        # y accumulation with M=65 single-tile matmuls whose 65th lhsT
        # column is all-ones -> row 64 of the y accumulator is the
        # softmax denominator (free: matmul time is N-bound).
            with tc.tile_pool(name="pp_qk", bufs=2, space="PSUM") as qk_pool, \
                 tc.tile_pool(name="att_s", bufs=2, space="PSUM") as s_pool, \
                 tc.tile_pool(name="att_y", bufs=2, space="PSUM") as y_pool, \
                 tc.tile_pool(name="att_es", bufs=4) as es_pool, \
                 tc.tile_pool(name="att_yr", bufs=4) as yr_pool, \
                 tc.tile_pool(name="att_st", bufs=4) as st_pool, \
                 tc.tile_pool(name="att_cl", bufs=3) as cl_pool, \
                 tc.tile_pool(name="att_rc", bufs=3) as rc_pool, \
                 tc.tile_pool(name="att_bc", bufs=4) as bc_pool:
                fence_ps = y_pool.tile([1, 8], F32, tag="y")
                nc.tensor.matmul(fence_ps[0:1, 0:1], ones_sb[0:1, 0:1],
                                 ones_sb[0:1, 0:1], start=True, stop=True)
                if use_bf16:
                    nc.tensor.matmul(fence_ps[0:1, 1:2], ones_f32[0:1, 0:1],
                                     ones_f32[0:1, 0:1], start=True, stop=True)
                for fi, ft in enumerate((xt0, wv_sb, xt1, wqk_sb, wp_sb)):
                    nc.tensor.matmul(fence_ps[0:1, 2 + fi:3 + fi],
                                     ft[0:1, 0, 0:1], ones_sb[0:1, 0:1],
                                     start=True, stop=True)
                for p in range(N_PAIR):
                    qk_proj(p, qk_pool)
                    yraw_a = yr_pool.tile([64, T], edt, tag="yraw")
                    yraw_b = yr_pool.tile([64, T], edt, tag="yraw")
                    yraws = [yraw_a, yraw_b]
                    for i in range(N_I):
                        isl = slice(i * I_BLK, (i + 1) * I_BLK)
                        y_a = y_pool.tile([65, I_BLK], F32, tag="y")
                        y_b = y_pool.tile([65, I_BLK], F32, tag="y")
                        ys = [y_a, y_b]
                        for j in range(NTC):
                            jsl = slice(j * 128, (j + 1) * 128)
                            s = s_pool.tile([128, 2 * I_BLK], F32, tag="s")
                            nc.tensor.matmul(s[:, 0:I_BLK],
                                             kT[0:64, p, jsl], qT[0:64, p, isl],
                                             start=True, stop=True)
                            nc.tensor.matmul(s[:, I_BLK:2 * I_BLK],
                                             kT[64:128, p, jsl],
                                             qT[64:128, p, isl],
                                             start=True, stop=True)
                            if p == 0 and i == 0:
                                v_proj_tc(j, qk_pool)
                            es = es_pool.tile([128, 2 * I_BLK], edt, tag="es")
                            nc.scalar.activation(es[:], s[:], EXP, scale=SCALE)
                            for hl in range(2):
                                h = 2 * p + hl
                                nc.tensor.matmul(
                                    ys[hl][0:65, :],
                                    v[:, j, 65 * h:65 * h + 65],
                                    es[:, hl * I_BLK:(hl + 1) * I_BLK],
                                    start=(j == 0), stop=(j == NTC - 1))
                        # per-i denominator handling so yT streams out
                        # (keeps the output projection from piling up at
                        # the very end of attention)
                        coll = cl_pool.tile([2, I_BLK], F32, tag="coll")
                        for hl in range(2):
                            st = st_pool.tile([65, I_BLK], F32, tag="st")
                            nc.vector.tensor_copy(st[64:65, :],
                                                  ys[hl][64:65, :])
                            nc.vector.tensor_copy(yraws[hl][:, isl],
                                                  ys[hl][0:64, :])
                            # move denominator row to its own partition
                            nc.gpsimd.dma_start(out=coll[hl:hl + 1, :],
                                                in_=st[64:65, :])
                        rc = rc_pool.tile([2, I_BLK], F32, tag="rc")
                        nc.vector.reciprocal_approx_fast(rc[:], coll[:])
                        nc.gpsimd.dma_start(out=rcd[p, 2 * i:2 * i + 2],
                                            in_=rc[:])
                        for hl in range(2):
                            bcast = bc_pool.tile([64, I_BLK], edt, tag="bcast")
                            rrow = rcd[p, 2 * i + hl, :]
                            rbc = bass.AP(tensor=rrow.tensor,
                                          offset=rrow.offset,
                                          ap=[[0, 64]] + list(rrow.ap))
                            nc.gpsimd.dma_start(out=bcast[:], in_=rbc)
                            if hl == 0:
                                nc.vector.tensor_mul(yT[0:64, p, isl],
                                                     yraws[0][:, isl],
                                                     bcast[:])
                            else:
                                ybst = bc_pool.tile([64, I_BLK], edt,
                                                    tag="ybst")
                                nc.vector.tensor_mul(ybst[:],
                                                     yraws[1][:, isl],
                                                     bcast[:])
                                nc.gpsimd.dma_start(out=yT[64:128, p, isl],
                                                    in_=ybst[:])

        # ---------------- phase 3: output projection ----------------
        with tc.tile_pool(name="ph3o", bufs=3) as ph3o, \
             tc.tile_pool(name="pp_o", bufs=3, space="PSUM") as pp_o:
            for tcl in range(NTC):
                ps = pp_o.tile([128, C], F32, tag="pso")
                for d in range(N_PAIR):
                    for n2 in range(C // 512):
                        nsl = slice(n2 * 512, (n2 + 1) * 512)
                        nc.tensor.matmul(
                            ps[:, nsl],
                            yT[:, d, tcl * 128:(tcl + 1) * 128],
                            wp_sb[:, d, nsl],
                            start=(d == 0), stop=(d == N_PAIR - 1))
                os = ph3o.tile([128, C], F32, tag="os")
                # absorb the WAR wait on the slot's previous out-DMA
                nc.vector.memset(os[0:1, 0:1], 0.0)
                nc.vector.tensor_copy(os[:], ps[:])
                nc.sync.dma_start(out=out[tcl * 128:(tcl + 1) * 128, :],
                                  in_=os[:])

    nc.compile()
    return nc


def make_in_maps(x, w_attn, b_attn, w_proj, T=T_FULL, use_bf16=True):
    """Host-side sharding: per-core input dict."""
    import ml_dtypes
    mdt = ml_dtypes.bfloat16 if use_bf16 else np.float32
    x = np.asarray(x, dtype=np.float32)
    w_attn = np.asarray(w_attn, dtype=np.float32)
    b_attn = np.asarray(b_attn, dtype=np.float32)
    w_proj = np.asarray(w_proj, dtype=np.float32)
    in_maps = []
    ones = np.ones((128, 64), dtype=np.float32)
    for core in range(N_CORES):
        b, g = core // 2, core % 2
        gq = slice(g * QKD, (g + 1) * QKD)
        gk = slice(C + g * QKD, C + (g + 1) * QKD)
        gv = slice(2 * C + g * QKD, 2 * C + (g + 1) * QKD)
        w_qk = np.concatenate([w_attn[:, gq], w_attn[:, gk]], axis=1)
        b_q = b_attn[gq]
        b_k = b_attn[gk]
        b_v = b_attn[gv]
        b_qk = np.stack([b_q.reshape(4, 128), b_k.reshape(4, 128)],
                        axis=0).reshape(8, 128).T.copy()   # [128, 8]
        in_maps.append({
            "xT": np.ascontiguousarray(x[b, :T].T).astype(mdt),
            "w_qk": np.ascontiguousarray(w_qk).astype(mdt),
            "w_v": np.ascontiguousarray(w_attn[:, gv]).astype(mdt),
            "b_qk": np.ascontiguousarray(b_qk),
            "b_v_bc": np.tile(b_v, (128, 1)),
            "w_pr": np.ascontiguousarray(w_proj[gq]).astype(mdt),
            "ones": ones,
        })
    return in_maps


def kernel(x, w_attn, b_attn, w_proj, b_proj):
    global LAST_RESULTS
    in_maps = make_in_maps(x, w_attn, b_attn, w_proj)
    nc = build_bass()
    res = run_bass_kernel_spmd(nc, in_maps, list(range(N_CORES)), trace=TRACE)
    LAST_RESULTS = res
    b_proj = np.asarray(b_proj, dtype=np.float32)
    out = np.empty((B, T_FULL, C), dtype=np.float32)
    for b in range(B):
        out[b] = res.results[2 * b]["out"] + res.results[2 * b + 1]["out"] \
            + b_proj[None, :]
    return out
